# revision 65
# baseline (speedup 1.0000x reference)
"""Trainium2 Bass kernel for the nn_EncoderBlock problem.

Full inputs in, full output out. 8-way SPMD: cores 0-3 handle batch 0,
cores 4-7 batch 1. Within each 4-core batch group, BOTH blocks are
query-sharded 4 ways (1024 owned rows per core). Between the blocks, the
post-LayerNorm activations zb2 = LN(LN(x2)) (bf16, the exact values the
matmuls would consume anyway) are exchanged with a DRAM AllGather over
replica groups [[0..3],[4..7]] so every core can build block-2 K/V for
the full sequence. Block-2 keys are consumed in global row order (the
gather's order); attention is permutation-invariant over keys, so this
coexists with each core's rotated local query order.

All 8 cores run the SAME program: the host rotates each core's token
order by its query offset -- "queries 0..1023" on the device are exactly
the core's own output shard, while the key set stays complete.

Per block: LN(LN(x)) -> QKV projections -> per-head attention with
scores kept transposed [keys, queries] so softmax's exp doubles as the
PSUM->SBUF evacuation on the scalar engine (no max-subtraction needed:
|scores| < 2), P*V via a ones-augmented V (M=65) so the softmax
denominator falls out of the same matmul, normalization via a K=1
outer-product broadcast matmul, output projection with bias folded in
as a K=1 matmul, residual add. bf16 matmul operands, f32 accumulation,
f32 residual stream. Score matmuls are row-packed two heads at a time
(K=64 pairs on array rows 0-63/64-127).
"""

import sys

sys.path.insert(0, "/opt/trn_rl_repo")

import numpy as np
import ml_dtypes

import bass_rust
import concourse.bass as bass
import concourse.tile as tile
from concourse import mybir
from concourse.bass_utils import run_bass_kernel_spmd

F32 = mybir.dt.float32
F32R = mybir.dt.float32r
BF16 = mybir.dt.bfloat16
FP8 = mybir.dt.float8e4
AF = mybir.ActivationFunctionType
ALU = mybir.AluOpType
DR = mybir.MatmulPerfMode.DoubleRow

P = 128
D = 384
H = 6
DK = 64
DT = D // P          # 3 D-chunks of 128
S = 4096             # full sequence per batch
NTS = S // P         # 32 token tiles of 128
NKC = S // P         # 32 key chunks of 128
OWN = 1024           # query tokens owned per core (both blocks)
NOT = OWN // P       # 8 owned token tiles
GRP = 4              # cores per batch group
EPS = 1e-6
QT = 512             # query tile (free dim of score matmuls)
NQT = OWN // QT      # 2 query tiles per block
KCG = 2              # key chunks per exp group (one fp8 DoubleRow pair);
                     # st shrinks to 2 PSUM banks so the pv accumulators
                     # get their own pool and attention can interleave
                     # into the K/V build phase
VROW = H * (DK + 1)  # 390: per-kc row of V_aug (64 data cols + ones col/head)
VROW2 = 400          # padded to a 16-byte-multiple stride for DoubleRow
CC_GROUPS = [[0, 1, 2, 3], [4, 5, 6, 7]]

# ---------------------------------------------------------------------------
# walrus in this container caps sync-waits per instruction (1 for most,
# 0 for DMA-transpose). Hoist excess waits onto same-engine NoOps.
_WAIT_LIMIT_BY_TYPE = {"InstDmaTransposeAnt": 0}
_wfix_ctr = [0]


def _fix_sync_waits(nc):
    for f in nc.m.functions:
        for bb in f.blocks:
            out = []
            changed = False
            for ins in bb.instructions:
                si = ins.sync_info
                waits = list(si.on_wait) if si is not None else []
                limit = _WAIT_LIMIT_BY_TYPE.get(type(ins).__name__, 1)
                if len(waits) > limit:
                    keep, hoist = waits[:limit], waits[limit:]
                    for w in hoist:
                        _wfix_ctr[0] += 1
                        nop = mybir.InstNoOp(
                            name=f"WFIX-{_wfix_ctr[0]}", engine=ins.engine
                        )
                        nop.sync_info = bass_rust.SyncInfo(on_wait=[w], on_update=[])
                        out.append(nop)
                    ins.sync_info = bass_rust.SyncInfo(
                        on_wait=keep, on_update=list(si.on_update)
                    )
                    changed = True
                out.append(ins)
            if changed:
                bb.instructions = out


def _rows(dram_ap, row0, nrows):
    """[nrows, D] rows of a [*, D] DRAM tensor as a DMA AP."""
    return bass.AP(tensor=dram_ap.tensor,
                   offset=dram_ap.offset + row0 * D,
                   ap=[[D, nrows], [1, D]])


# ---------------------------------------------------------------------------
def _emit_ln_tile(nc, pools, C, x_src_d, n, blk, src_sb=None):
    """LN(LN(x)) for one 128-token tile -> zb (bf16). Pure DVE + tiny ACT,
    zero PSUM usage, so these interleave into attention without stalling
    the in-order PE stream.

    When every LN affine is identity (the staged problem: a=1, b=0),
    LN(LN(x)) folds exactly to (x - m) / (s*(1+eps) + eps^2): the inner
    LN's output has mean 0 and std s/(s+eps), so the outer divide just
    rescales the same centered row. One bn_stats + one tensor_scalar
    per tile instead of two full passes."""
    work = pools["work"]

    if C["identity_ln"]:
        if src_sb is not None:
            xt = src_sb
        else:
            xt = work.tile([P, D], F32, tag="x_ln", bufs=8,
                           name=f"xln{blk}_{n}")
            # first chunk rides the (empty) ACT HWDGE queue instead of
            # queueing behind the constant-weight DMAs on SP
            eng = nc.scalar if (blk == 0 and n < 8) else nc.sync
            eng.dma_start(out=xt[:, :], in_=_rows(x_src_d, n * P, P))
        mv = work.tile([P, 6 + 2], F32, tag="ln_mv", bufs=8,
                       name=f"mvf_{blk}_{n}")
        nc.vector.bn_stats(out=mv[:, 0:6], in_=xt[:, :])
        nc.vector.bn_aggr(out=mv[:, 6:8], in_=mv[:, 0:6])
        r = work.tile([P, 1], F32, tag="ln_r", bufs=8,
                      name=f"rf_{blk}_{n}")
        nc.scalar.activation(out=r[:, :], in_=mv[:, 7:8], func=AF.Sqrt,
                             scale=float(D) / float(D - 1))
        nc.vector.tensor_scalar(out=r[:, :], in0=r[:, :],
                                scalar1=1.0 + EPS, scalar2=EPS * EPS,
                                op0=ALU.mult, op1=ALU.add)
        nc.vector.reciprocal(out=r[:, :], in_=r[:, :])
        zb = work.tile([P, D], BF16, tag=f"zb{blk}",
                       bufs=(NOT if blk == 1 else 8),
                       name=f"zb_{blk}_{n}")
        nc.vector.tensor_scalar(
            out=zb[:, :], in0=xt[:, :],
            scalar1=mv[:, 6:7], scalar2=r[:, 0:1],
            op0=ALU.subtract, op1=ALU.mult)
        return zb

    def _ln_pass(src_ap, m_ra, m_rb, dst_ap, uid):
        mv = work.tile([P, 6 + 2], F32, tag="ln_mv", name=f"mv_{uid}")
        nc.vector.bn_stats(out=mv[:, 0:6], in_=src_ap)
        nc.vector.bn_aggr(out=mv[:, 6:8], in_=mv[:, 0:6])
        r = work.tile([P, 1], F32, tag="ln_r", name=f"r_{uid}")
        nc.scalar.activation(out=r[:, :], in_=mv[:, 7:8], func=AF.Ln,
                             scale=float(D) / float(D - 1))
        nc.scalar.activation(out=r[:, :], in_=r[:, :], func=AF.Exp,
                             scale=0.5)
        nc.vector.tensor_scalar_add(out=r[:, :], in0=r[:, :], scalar1=EPS)
        nc.vector.reciprocal(out=r[:, :], in_=r[:, :])
        t = work.tile([P, D], F32, tag="ln_t", name=f"t_{uid}")
        nc.vector.tensor_scalar(
            out=t[:, :], in0=src_ap,
            scalar1=mv[:, 6:7], scalar2=r[:, 0:1],
            op0=ALU.subtract, op1=ALU.mult)
        nc.vector.tensor_mul(out=t[:, :], in0=t[:, :], in1=m_ra[:, :])
        nc.vector.tensor_add(out=dst_ap, in0=t[:, :], in1=m_rb[:, :])

    if src_sb is not None:
        xt = src_sb
    else:
        xt = work.tile([P, D], F32, tag="x_ln", name=f"xln{blk}_{n}")
        nc.sync.dma_start(out=xt[:, :], in_=_rows(x_src_d, n * P, P))
    yt = work.tile([P, D], F32, tag="y1", name=f"y1_{blk}_{n}")
    _ln_pass(xt[:, :], C["ra_bc"], C["rb_bc"], yt[:, :], f"{blk}_{n}a")
    zb = work.tile([P, D], BF16, tag=f"zb{blk}",
                   bufs=(NOT if blk == 1 else 4),
                   name=f"zb_{blk}_{n}")
    _ln_pass(yt[:, :], C["a0_bc"], C["b0_bc"], zb[:, :], f"{blk}_{n}b")
    return zb


def _transpose_tile(nc, pools, C, zb_ap, dst, col0, uid):
    """3 PE transposes of a [128, D] bf16 SBUF tile into dst[:, :, col0:..].
    PSUM->SBUF evacuation on ACT: it is idle in the LN valley / boundary,
    while DVE is the critical engine there. (XBAR DMA transposes are only
    used for the big DRAM gather halves: their 0-wait walrus limit turns
    every dependency into a queue-blocking NoOp, which serializes the LN
    pipeline when used per-tile.)"""
    psB = pools["psB"]
    for dt_ in range(DT):
        tp = psB.tile([P, P], BF16, tag="acc", name=f"tp{uid}_{dt_}")
        nc.tensor.transpose(out=tp[:, 0:P],
                            in_=zb_ap[:, dt_ * P:(dt_ + 1) * P],
                            identity=C["ident"][:, :])
        nc.vector.tensor_copy(out=dst[:, dt_, col0:col0 + P], in_=tp[:, 0:P])


# ---------------------------------------------------------------------------
def _build_block(nc, pools, C, x_src_d, out_d, blk,
                 zb_dram=None, premade_zb=None, after_qt=None,
                 out_sb=None, res_sb=None):
    """One residual MSA block, query-sharded to OWN rows.

    x_src_d: DRAM AP [*, D] f32 -- local rows (rotated order); rows
             0..OWN are this core's queries and residual base.
    out_d:   DRAM AP [OWN, D] f32 -- gets x_src[0:OWN] + MSA(...)[0:OWN]
             (ignored when out_sb is given).
    zb_dram: if set (block 2), two [S//2, D] bf16 DRAM halves with
             LN(LN(x2)) for the WHOLE sequence in gather order -- K/V
             source. If None (block 1), LN chains run locally.
    premade_zb: block 2 only -- the 8 locally-computed own zb tiles
             (rotated order), transposed into zTq for the Q projection.
    out_sb:  block 1 -- list that receives the 8 SBUF residual tiles
             instead of writing them to DRAM.
    res_sb:  block 2 -- those same tiles, used as the residual base.
    """
    work, psA, psB, psV, ste_pool, otp = (pools[k] for k in
                                          ("work", "psA", "psB", "psV",
                                           "ste", "ot"))

    qt_sb, kt_sb, v_aug = C["qt"], C["kt"], C["v_aug"]
    zTh = C["zTh"]          # two [P, DT, S//2] halves
    HNT = NTS // 2          # 16 token tiles per half

    def _q_proj(src):
        for dt_ in range(DT):
            for ntk in range(NQT):
                ps = psB.tile([P, QT], F32, tag="acc",
                              name=f"pq{blk}_{dt_}_{ntk}")
                for ki in range(DT):
                    nc.tensor.matmul(
                        ps[:, :],
                        lhsT=C["wqT"][:, ki, dt_ * P:(dt_ + 1) * P],
                        rhs=src[:, ki, ntk * QT:(ntk + 1) * QT],
                        start=(ki == 0), stop=(ki == DT - 1))
                nc.vector.tensor_scalar(
                    out=qt_sb[:, dt_, ntk * QT:(ntk + 1) * QT],
                    in0=ps[:, :], scalar1=C["bq_col"][:, dt_:dt_ + 1],
                    scalar2=None, op0=ALU.add)

    def _kv_z(n0, n1):
        """zT from local LN chains for token tiles n0..n1 (block 1)."""
        for n in range(n0, n1):
            zb = _emit_ln_tile(nc, pools, C, x_src_d, n, blk)
            _transpose_tile(nc, pools, C, zb[:, :], zTh[n // HNT],
                            (n % HNT) * P, f"z{blk}_{n}")

    def _kv_z_half(h):
        """zT half h from the gathered DRAM buffer (block 2): one XBAR
        transpose; on the SP queue its wait-NoOps only delay later
        residual loads."""
        nc.sync.dma_start_transpose(zTh[h][:, :, :], zb_dram[h])

    def _kv_proj(n0, n1):
        """K-proj + V_aug for token tiles n0..n1 (multiple of 4)."""
        for dt_ in range(DT):
            for ntk in range(n0 * P // QT, n1 * P // QT):
                zt_src = zTh[ntk * QT // (S // 2)]
                col = ntk * QT % (S // 2)
                ps = psB.tile([P, QT], F32, tag="acc",
                              name=f"pk{blk}_{dt_}_{ntk}")
                for ki in range(DT):
                    nc.tensor.matmul(
                        ps[:, :],
                        lhsT=C["wkT"][:, ki, dt_ * P:(dt_ + 1) * P],
                        rhs=zt_src[:, ki, col:col + QT],
                        start=(ki == 0), stop=(ki == DT - 1))
                nc.vector.tensor_scalar(
                    out=kt_sb[:, dt_, ntk * QT:(ntk + 1) * QT],
                    in0=ps[:, :], scalar1=C["bk_col"][:, dt_:dt_ + 1],
                    scalar2=None, op0=ALU.add)
        for n in range(n0, n1):
            ps = psB.tile([P, QT], F32, tag="acc", name=f"v{blk}_{n}")
            for ki in range(DT):
                nc.tensor.matmul(
                    ps[:, :D],
                    lhsT=zTh[n // HNT][:, ki, (n % HNT) * P:(n % HNT + 1) * P],
                    rhs=C["wvT"][:, ki, :],
                    start=(ki == 0), stop=(ki == DT - 1))
            # ones-augmented layout; ones at j=DK persist from memset
            v_dst = v_aug[:, n, 0:VROW].rearrange(
                "p (h j) -> p h j", h=H, j=DK + 1)[:, :, 0:DK]
            if C["zero_bv"]:
                nc.scalar.copy(
                    out=v_dst,
                    in_=ps[:, :D].rearrange("p (h j) -> p h j", h=H, j=DK))
            else:
                nc.vector.tensor_tensor(
                    out=v_dst,
                    in0=ps[:, :D].rearrange("p (h j) -> p h j", h=H, j=DK),
                    in1=C["bv_bc"][:, :].rearrange("p (h j) -> p h j",
                                                   h=H, j=DK),
                    op=ALU.add)

    # ---- attention helpers (KCG=2: one fp8 DoubleRow pair per group) ----
    NG = NKC // KCG

    def _emit_group(ntk, hp, pv, g):
        kcs = list(range(g * KCG, (g + 1) * KCG))
        w = KCG * QT
        # both halves' score matmuls first (complementary tile_positions
        # on PE rows 0-63/64-127), then the exps, then the P*V consumers.
        st2, ste2 = [], []
        for half in range(2):       # head pair on partitions 0-63/64-127
            lo = half * DK
            st = psA.tile([P, KCG * QT], F32, tag="st",
                          name=f"st{blk}_{ntk}_{hp}_{g}_{half}")
            st2.append(st)
            for j, kc in enumerate(kcs):
                nc.tensor.matmul(
                    st[:, j * QT:(j + 1) * QT],
                    lhsT=kt_sb[lo:lo + DK, hp, kc * P:(kc + 1) * P],
                    rhs=qt_sb[lo:lo + DK, hp, ntk * QT:(ntk + 1) * QT],
                    start=True, stop=True)
        for half in range(2):
            ste = ste_pool.tile([P, KCG * QT], FP8, tag="ste",
                                name=f"se{blk}_{ntk}_{hp}_{g}_{half}")
            ste2.append(ste)
            nc.scalar.activation(out=ste[:, :w], in_=st2[half][:, :w],
                                 func=AF.Exp, scale=1.0 / 8.0)
        for half in range(2):
            h = 2 * hp + half
            kc = kcs[0]
            nc.tensor.matmul(
                pv[half][0:DK + 1, :],
                lhsT=v_aug[:, kc:kc + 2,
                           h * (DK + 1):(h + 1) * (DK + 1)],
                rhs=ste2[half][:, :].rearrange(
                    "p (k q) -> p k q", k=KCG, q=QT),
                start=(kc == 0), stop=(kc + 1 == NKC - 1),
                perf_mode=DR, skip_group_check=True)

    def _new_pv(ntk, hp):
        return [psV.tile([P, QT], F32, tag="pv",
                         name=f"pv{blk}_{ntk}_{hp}_{i}") for i in range(2)]

    def _hp_tail(ntk, hp, pv, ot):
        for half in range(2):
            lo = half * DK
            r_row = work.tile([1, QT], F32R, tag="r_row",
                              name=f"rr{blk}_{ntk}_{hp}_{half}")
            with nc.allow_low_precision(
                    reason="f32r broadcast of softmax denom"):
                nc.vector.reciprocal(
                    out=r_row[:, :], in_=pv[half][DK:DK + 1, :])
            # from psB (idle during the hp tails) so the st rotation isn't
            # blocked and the next hp's scores can start immediately
            r_bc = psB.tile([P, QT], F32, tag="acc",
                            name=f"rb{blk}_{ntk}_{hp}_{half}")
            nc.tensor.matmul(
                r_bc[0:DK, 0:QT],
                lhsT=C["ones"][0:1, 0:DK],
                rhs=r_row[0:1, :],
                start=True, stop=True)
            r_sb = work.tile([DK, QT], F32, tag="r_sb",
                             name=f"rs{blk}_{ntk}_{hp}_{half}")
            nc.vector.tensor_copy(out=r_sb[:, :], in_=r_bc[0:DK, 0:QT])
            nc.vector.tensor_tensor(
                out=ot[lo:lo + DK, hp, :],
                in0=pv[half][0:DK, :], in1=r_sb[:, :], op=ALU.mult)

    def _emit_out(ntk, ot):
        """Output projection + bias + residual for query tile ntk.

        Block 1 keeps the residual-stream tiles in SBUF (out_sb) -- no
        DRAM roundtrip; block 2 reads its residual base straight from
        those retained tiles and DMAs the final rows out.
        """
        for c4 in range(QT // P):
            tok = ntk * QT + c4 * P
            ps = psB.tile([P, QT], F32, tag="acc",
                          name=f"o{blk}_{ntk}_{c4}")
            for ki in range(DT):
                nc.tensor.matmul(
                    ps[:, :D],
                    lhsT=ot[:, ki, c4 * P:(c4 + 1) * P],
                    rhs=C["woT"][:, ki, :],
                    start=(ki == 0), stop=False)
            nc.tensor.matmul(
                ps[:, :D],
                lhsT=C["ones"][0:1, 0:P],
                rhs=C["bo_row"][0:1, :],
                start=False, stop=True, skip_group_check=True)
            if res_sb is not None:
                xr = res_sb[tok // P]
            else:
                xr = work.tile([P, D], F32, tag="x_res",
                               name=f"xr{blk}_{ntk}_{c4}")
                nc.sync.dma_start(out=xr[:, :], in_=_rows(x_src_d, tok, P))
            xo = work.tile([P, D], F32, tag=f"x_out{blk}",
                           bufs=(NOT if out_sb is not None else 3),
                           name=f"xo{blk}_{ntk}_{c4}")
            nc.vector.tensor_tensor(
                out=xo[:, :], in0=ps[:, :D], in1=xr[:, :], op=ALU.add)
            if out_sb is not None:
                out_sb[tok // P] = xo
            else:
                nc.sync.dma_start(out=_rows(out_d, tok, P), in_=xo[:, :])
        if after_qt is not None:
            after_qt(ntk)

    # ---- build: K/V interleaved with the first attention pass, so ACT
    # gets exp work while the front/boundary phases run ----
    ot0 = otp.tile([P, DT, QT], BF16, tag="ot", name=f"ot{blk}_0")
    pv0 = _new_pv(0, 0)
    if premade_zb is None:
        # block 1: all LN local; zTh[0] cols 0..OWN are exactly the own
        # queries. After 8 tiles the Q projection runs, then attention
        # groups (ntk=0, hp=0) chase the K/V chunks tile-availability.
        # Projections lag the LN stream one chunk so DVE's in-order queue
        # never parks a PSUM evacuation in front of the next chunk's LN.
        NCH = NTS // 4
        _kv_z(0, 4)
        _kv_z(4, 8), _kv_proj(0, 4)
        _q_proj(zTh[0])
        g_done = 0
        for c in range(2, NCH + 1):
            if c < NCH:
                _kv_z(4 * c, 4 * c + 4)
            _kv_proj(4 * (c - 1), 4 * c)
            g_av = min(2 * c, NG)
            for g in range(g_done, g_av):
                _emit_group(0, 0, pv0, g)
            g_done = g_av
    elif True:
        # block 2: gather0-dependent K/V first (ready the moment block 1
        # ends); the Q path was already emitted inside block 1 via the
        # after_qt callbacks (its ntk-0 half lands mid-block-1). While
        # gather1 is in flight, ALL THREE head-pairs process the half-0
        # keys -- partial pv accumulators spill to SBUF between head-pairs
        # so two PSUM banks suffice -- then reload and finish once half-1
        # lands.
        NGH = NG // 2
        _kv_z_half(0)
        for c in range(0, 4):
            _kv_proj(4 * c, 4 * c + 4)
        spills = []
        for hp in range(DT):
            pv = pv0 if hp == 0 else _new_pv(0, hp)
            for g in range(NGH):
                _emit_group(0, hp, pv, g)
            sb = work.tile([P, 2, QT], F32, tag="pvsp", bufs=3,
                           name=f"pvsp{hp}")
            for half in range(2):
                nc.vector.tensor_copy(out=sb[0:DK + 1, half, :],
                                      in_=pv[half][0:DK + 1, :])
            spills.append(sb)
        # psB fence: the Tile scheduler models the collective as ~free, so
        # without a data dependency it sprinkles the gather1-gated
        # projection matmuls into the PE queue AHEAD of the ready
        # spill-phase attention, head-blocking the engine for the whole
        # gather. Route both psB slots through dummies that depend on the
        # last head-pair's spill so those matmuls cannot be hoisted.
        fsc = work.tile([P, 2], F32, tag="fsc", name="fsc")
        for i in range(2):
            fce = psB.tile([P, QT], F32, tag="acc", name=f"fence{i}")
            nc.vector.tensor_copy(out=fce[0:1, 0:1],
                                  in_=spills[2][0:1, 1, 0:1])
            nc.vector.tensor_copy(out=fsc[0:1, i:i + 1], in_=fce[0:1, 0:1])
        _kv_z_half(1)
        for c in range(4, NTS // 4):
            _kv_proj(4 * c, 4 * c + 4)
        for hp in range(DT):
            pv = _new_pv(0, hp)
            for half in range(2):
                nc.vector.tensor_copy(out=pv[half][0:DK + 1, :],
                                      in_=spills[hp][0:DK + 1, half, :])
            for g in range(NGH, NG):
                _emit_group(0, hp, pv, g)
            _hp_tail(0, hp, pv, ot0)
        _emit_out(0, ot0)
    if premade_zb is None:
        _hp_tail(0, 0, pv0, ot0)
        for hp in range(1, DT):
            pv = _new_pv(0, hp)
            for g in range(NG):
                _emit_group(0, hp, pv, g)
            _hp_tail(0, hp, pv, ot0)
        _emit_out(0, ot0)
    for ntk in range(1, NQT):
        ot = otp.tile([P, DT, QT], BF16, tag="ot", name=f"ot{blk}_{ntk}")
        for hp in range(DT):
            pv = _new_pv(ntk, hp)
            for g in range(NG):
                _emit_group(ntk, hp, pv, g)
            _hp_tail(ntk, hp, pv, ot)
        _emit_out(ntk, ot)


def _build_program(identity_ln, zero_bv=True):
    nc = bass.Bass("TRN2", target_bir_lowering=False, debug=False,
                   num_devices=8)

    di = {}
    di["xs"] = nc.dram_tensor("xs", [S, D], F32, kind="ExternalInput")
    for w in ("wqT", "wkT", "wvT", "woT"):
        di[w] = nc.dram_tensor(w, [D, D], BF16, kind="ExternalInput")
    di["bq_col"] = nc.dram_tensor("bq_col", [P, DT], F32, kind="ExternalInput")
    di["bk_col"] = nc.dram_tensor("bk_col", [P, DT], F32, kind="ExternalInput")
    di["bv_bc"] = nc.dram_tensor("bv_bc", [P, D], F32, kind="ExternalInput")
    di["bo_row"] = nc.dram_tensor("bo_row", [1, D], F32R, kind="ExternalInput")
    for w in ("ra0_bc", "rb0_bc", "ra1_bc", "rb1_bc", "a0_bc", "b0_bc"):
        di[w] = nc.dram_tensor(w, [P, D], F32, kind="ExternalInput")
    di["ones_in"] = nc.dram_tensor("ones_in", [1, P], F32R,
                                   kind="ExternalInput")
    di["ident_in"] = nc.dram_tensor("ident_in", [P, P], BF16,
                                    kind="ExternalInput")
    out_d = nc.dram_tensor("out", [OWN, D], F32, kind="ExternalOutput")
    # split gather: half h carries each member's own rows [h*512:(h+1)*512];
    # separate tensors so first-half consumers never falsely depend on the
    # second collective.
    gin_d = [nc.dram_tensor(f"gin{h}", [OWN // 2, D], BF16) for h in range(2)]
    gout_d = [nc.dram_tensor(f"gout{h}", [S // 2, D], BF16) for h in range(2)]

    with tile.TileContext(nc) as tc:
        with tc.tile_pool(name="const", bufs=1) as const, \
             tc.tile_pool(name="work", bufs=3) as work, \
             tc.tile_pool(name="ot", bufs=2) as otp, \
             tc.tile_pool(name="ste", bufs=6) as ste_pool, \
             tc.tile_pool(name="psA", bufs=2, space="PSUM") as psA, \
             tc.tile_pool(name="psB", bufs=2, space="PSUM") as psB, \
             tc.tile_pool(name="psV", bufs=2, space="PSUM") as psV:

            pools = {"work": work, "psA": psA, "psB": psB, "psV": psV,
                     "ste": ste_pool, "ot": otp}

            C = {}
            for wname in ("wqT", "wkT", "wvT", "woT"):
                C[wname] = const.tile([P, DT, D], BF16, name=wname)
                nc.sync.dma_start(
                    out=C[wname][:, :, :],
                    in_=di[wname][:].rearrange("(d p) e -> p d e", p=P))
            for wname in ("bq_col", "bk_col", "bv_bc"):
                C[wname] = const.tile(list(di[wname].shape), F32, name=wname)
                nc.sync.dma_start(out=C[wname][:], in_=di[wname][:])
            C["bo_row"] = const.tile([1, D], F32R, name="bo_row")
            nc.sync.dma_start(out=C["bo_row"][:], in_=di["bo_row"][:])
            for wname in ("ra0_bc", "rb0_bc", "ra1_bc", "rb1_bc",
                          "a0_bc", "b0_bc"):
                C[wname] = const.tile([P, D], F32, name=wname)
                nc.sync.dma_start(out=C[wname][:, :], in_=di[wname][:])
            C["ones"] = const.tile([1, P], F32R, name="ones")
            nc.sync.dma_start(out=C["ones"][:, :], in_=di["ones_in"][:])
            C["ident"] = const.tile([P, P], BF16, name="ident")
            nc.sync.dma_start(out=C["ident"][:, :], in_=di["ident_in"][:])

            C["zTh"] = [const.tile([P, DT, S // 2], BF16, name=f"zT{h}")
                        for h in range(2)]
            C["zTq"] = const.tile([P, DT, OWN], BF16, name="zTq")
            C["qt"] = const.tile([P, DT, OWN], BF16, name="qt")
            C["kt"] = const.tile([P, DT, S], BF16, name="kt")
            C["v_aug"] = const.tile([P, NTS, VROW2], FP8, name="v_aug")
            nc.vector.memset(C["v_aug"][:, :, :], 1.0)

            C["identity_ln"] = identity_ln
            C["zero_bv"] = zero_bv
            C1 = dict(C)
            C1["ra_bc"], C1["rb_bc"] = C["ra0_bc"], C["rb0_bc"]
            C2 = dict(C)
            C2["ra_bc"], C2["rb_bc"] = C["ra1_bc"], C["rb1_bc"]

            # block-2 LN of OWN rows interleaves into block-1 attention:
            # as each query tile's x2 rows land (in SBUF -- no DRAM
            # roundtrip), run the pure-DVE LN chain, stage the bf16 result
            # into that half's gather input, and kick the half-AllGather
            # that publishes it group-wide. The first gather runs
            # concurrently with block-1's second query tile.
            zb2 = [None] * NOT
            x2_sb = [None] * NOT

            def _b2_q(ntk):
                """Block-2 Q path for query tile ntk: zTq transposes of the
                own zb2 tiles plus the Q projection. Emitted from inside
                block 1 so the ntk-0 half is ready mid-block-1 and block-2
                attention can start the instant block 1 drains."""
                for n in range(ntk * 4, ntk * 4 + 4):
                    _transpose_tile(nc, pools, C, zb2[n][:, :], C["zTq"],
                                    n * P, f"q1_{n}")
                for dt_ in range(DT):
                    ps = psB.tile([P, QT], F32, tag="acc",
                                  name=f"pq1_{dt_}_{ntk}")
                    for ki in range(DT):
                        nc.tensor.matmul(
                            ps[:, :],
                            lhsT=C["wqT"][:, ki, dt_ * P:(dt_ + 1) * P],
                            rhs=C["zTq"][:, ki, ntk * QT:(ntk + 1) * QT],
                            start=(ki == 0), stop=(ki == DT - 1))
                    nc.vector.tensor_scalar(
                        out=C["qt"][:, dt_, ntk * QT:(ntk + 1) * QT],
                        in0=ps[:, :], scalar1=C["bq_col"][:, dt_:dt_ + 1],
                        scalar2=None, op0=ALU.add)

            def _b2_ln(ntk):
                for n in range(ntk * 4, ntk * 4 + 4):
                    zb2[n] = _emit_ln_tile(nc, pools, C2, None, n, 1,
                                           src_sb=x2_sb[n])
                    nc.sync.dma_start(
                        out=_rows(gin_d[ntk][:], (n % 4) * P, P),
                        in_=zb2[n][:, :])
                nc.gpsimd.collective_compute(
                    "AllGather", mybir.AluOpType.bypass,
                    replica_groups=CC_GROUPS,
                    ins=[gin_d[ntk][:].opt()], outs=[gout_d[ntk][:].opt()])
                _b2_q(ntk)

            _build_block(nc, pools, C1, di["xs"][:], None, 0,
                         after_qt=_b2_ln, out_sb=x2_sb)
            _build_block(nc, pools, C2, None, out_d[:], 1,
                         zb_dram=[g[:] for g in gout_d], premade_zb=zb2,
                         res_sb=x2_sb)

    _fix_sync_waits(nc)
    return nc


_NC_CACHE = {}


def _get_nc(identity_ln=True, zero_bv=True):
    key = (identity_ln, zero_bv)
    if key not in _NC_CACHE:
        _NC_CACHE[key] = _build_program(identity_ln, zero_bv)
    return _NC_CACHE[key]


def _prep_inputs(x, a0, b0, ra0, rb0, ra1, rb1,
                 wq, bq, wk, bk, wv, bv, wo, bo):
    bf = ml_dtypes.bfloat16
    base = {
        "wqT": np.ascontiguousarray(np.asarray(wq, np.float32).T).astype(bf),
        "wkT": np.ascontiguousarray(np.asarray(wk, np.float32).T).astype(bf),
        "wvT": np.ascontiguousarray(np.asarray(wv, np.float32).T).astype(bf),
        "woT": np.ascontiguousarray(np.asarray(wo, np.float32).T).astype(bf),
        "bq_col": np.ascontiguousarray(
            np.asarray(bq, np.float32).reshape(DT, P).T),
        "bk_col": np.ascontiguousarray(
            np.asarray(bk, np.float32).reshape(DT, P).T),
        "bv_bc": np.ascontiguousarray(
            np.broadcast_to(np.asarray(bv, np.float32), (P, D))),
        "bo_row": np.asarray(bo, np.float32).reshape(1, D).copy(),
        "ra0_bc": np.ascontiguousarray(
            np.broadcast_to(np.asarray(ra0, np.float32), (P, D))),
        "rb0_bc": np.ascontiguousarray(
            np.broadcast_to(np.asarray(rb0, np.float32), (P, D))),
        "ra1_bc": np.ascontiguousarray(
            np.broadcast_to(np.asarray(ra1, np.float32), (P, D))),
        "rb1_bc": np.ascontiguousarray(
            np.broadcast_to(np.asarray(rb1, np.float32), (P, D))),
        "a0_bc": np.ascontiguousarray(
            np.broadcast_to(np.asarray(a0, np.float32), (P, D))),
        "b0_bc": np.ascontiguousarray(
            np.broadcast_to(np.asarray(b0, np.float32), (P, D))),
        "ones_in": np.ones((1, P), np.float32),
        "ident_in": np.eye(P, dtype=np.float32).astype(bf),
    }
    x = np.asarray(x, np.float32)
    in_maps = []
    for c in range(8):
        b, q0 = c // GRP, (c % GRP) * OWN
        m = dict(base)
        # rotate tokens so this core's output shard sits at rows 0..OWN
        m["xs"] = np.ascontiguousarray(
            np.concatenate([x[b, q0:], x[b, :q0]], axis=0))
        in_maps.append(m)
    return in_maps


def kernel(**inputs):
    identity_ln = all(
        bool(np.all(np.asarray(inputs[k], np.float32) == v))
        for k, v in (("a0", 1.0), ("b0", 0.0), ("ra0", 1.0), ("rb0", 0.0),
                     ("ra1", 1.0), ("rb1", 0.0)))
    zero_bv = all(
        bool(np.all(np.asarray(inputs[k], np.float32) == 0.0))
        for k in ("bq", "bk", "bv"))
    nc = _get_nc(identity_ln, zero_bv)
    in_maps = _prep_inputs(**inputs)
    res = run_bass_kernel_spmd(nc, in_maps, list(range(8)))
    B = inputs["x"].shape[0]
    out = np.empty((B, S, D), np.float32)
    for c in range(8):
        b, q0 = c // GRP, (c % GRP) * OWN
        out[b, q0:q0 + OWN, :] = res.results[c]["out"]
    return out


if __name__ == "__main__":
    rng = np.random.default_rng(0)
    ins = {
        "x": rng.standard_normal((2, S, D)).astype(np.float32),
        "a0": np.ones(D, np.float32), "b0": np.zeros(D, np.float32),
        "ra0": np.ones(D, np.float32), "rb0": np.zeros(D, np.float32),
        "ra1": np.ones(D, np.float32), "rb1": np.zeros(D, np.float32),
        "wq": (rng.standard_normal((D, D)) * 0.02).astype(np.float32),
        "bq": np.zeros(D, np.float32),
        "wk": (rng.standard_normal((D, D)) * 0.02).astype(np.float32),
        "bk": np.zeros(D, np.float32),
        "wv": (rng.standard_normal((D, D)) * 0.02).astype(np.float32),
        "bv": np.zeros(D, np.float32),
        "wo": (rng.standard_normal((D, D)) * 0.02).astype(np.float32),
        "bo": np.zeros(D, np.float32),
    }
    out = kernel(**ins)
    print("kernel ran, out shape", out.shape, out.dtype)


# revision 86
# speedup vs baseline: 1.0421x; 1.0421x over previous
"""Trainium2 Bass kernel for the nn_EncoderBlock problem.

Full inputs in, full output out. 8-way SPMD: cores 0-3 handle batch 0,
cores 4-7 batch 1. Within each 4-core batch group, BOTH blocks are
query-sharded 4 ways (1024 owned rows per core). Between the blocks, the
post-LayerNorm activations zb2 = LN(LN(x2)) (bf16, the exact values the
matmuls would consume anyway) are exchanged with a DRAM AllGather over
replica groups [[0..3],[4..7]] so every core can build block-2 K/V for
the full sequence. Block-2 keys are consumed in global row order (the
gather's order); attention is permutation-invariant over keys, so this
coexists with each core's rotated local query order.

All 8 cores run the SAME program: the host rotates each core's token
order by its query offset -- "queries 0..1023" on the device are exactly
the core's own output shard, while the key set stays complete.

Per block: LN(LN(x)) -> QKV projections -> per-head attention with
scores kept transposed [keys, queries] so softmax's exp doubles as the
PSUM->SBUF evacuation on the scalar engine (no max-subtraction needed:
|scores| < 2), P*V via a ones-augmented V (M=65) so the softmax
denominator falls out of the same matmul, normalization via a K=1
outer-product broadcast matmul, output projection with bias folded in
as a K=1 matmul, residual add. bf16 matmul operands, f32 accumulation,
f32 residual stream. Score matmuls are row-packed two heads at a time
(K=64 pairs on array rows 0-63/64-127).
"""

import sys

sys.path.insert(0, "/opt/trn_rl_repo")

import numpy as np
import ml_dtypes

import bass_rust
import concourse.bass as bass
import concourse.tile as tile
from concourse import mybir
from concourse.bass_utils import run_bass_kernel_spmd

F32 = mybir.dt.float32
F32R = mybir.dt.float32r
BF16 = mybir.dt.bfloat16
FP8 = mybir.dt.float8e4
AF = mybir.ActivationFunctionType
ALU = mybir.AluOpType
DR = mybir.MatmulPerfMode.DoubleRow

P = 128
D = 384
H = 6
DK = 64
DT = D // P          # 3 D-chunks of 128
S = 4096             # full sequence per batch
NTS = S // P         # 32 token tiles of 128
NKC = S // P         # 32 key chunks of 128
OWN = 1024           # query tokens owned per core (both blocks)
NOT = OWN // P       # 8 owned token tiles
GRP = 4              # cores per batch group
EPS = 1e-6
QT = 512             # query tile (free dim of score matmuls)
NQT = OWN // QT      # 2 query tiles per block
KCG = 2              # key chunks per exp group (one fp8 DoubleRow pair);
                     # st shrinks to 2 PSUM banks so the pv accumulators
                     # get their own pool and attention can interleave
                     # into the K/V build phase
VROW = H * (DK + 1)  # 390: per-kc row of V_aug (64 data cols + ones col/head)
VROW2 = 400          # padded to a 16-byte-multiple stride for DoubleRow
CC_GROUPS = [[0, 1, 2, 3], [4, 5, 6, 7]]

# ---------------------------------------------------------------------------
# walrus in this container caps sync-waits per instruction (1 for most,
# 0 for DMA-transpose). Hoist excess waits onto same-engine NoOps.
_WAIT_LIMIT_BY_TYPE = {"InstDmaTransposeAnt": 0}
_wfix_ctr = [0]


def _fix_sync_waits(nc):
    for f in nc.m.functions:
        for bb in f.blocks:
            out = []
            changed = False
            for ins in bb.instructions:
                si = ins.sync_info
                waits = list(si.on_wait) if si is not None else []
                limit = _WAIT_LIMIT_BY_TYPE.get(type(ins).__name__, 1)
                if len(waits) > limit:
                    keep, hoist = waits[:limit], waits[limit:]
                    for w in hoist:
                        _wfix_ctr[0] += 1
                        nop = mybir.InstNoOp(
                            name=f"WFIX-{_wfix_ctr[0]}", engine=ins.engine
                        )
                        nop.sync_info = bass_rust.SyncInfo(on_wait=[w], on_update=[])
                        out.append(nop)
                    ins.sync_info = bass_rust.SyncInfo(
                        on_wait=keep, on_update=list(si.on_update)
                    )
                    changed = True
                out.append(ins)
            if changed:
                bb.instructions = out


def _rows(dram_ap, row0, nrows):
    """[nrows, D] rows of a [*, D] DRAM tensor as a DMA AP."""
    return bass.AP(tensor=dram_ap.tensor,
                   offset=dram_ap.offset + row0 * D,
                   ap=[[D, nrows], [1, D]])


# ---------------------------------------------------------------------------
def _emit_ln_tile(nc, pools, C, x_src_d, n, blk, src_sb=None):
    """LN(LN(x)) for one 128-token tile -> zb (bf16). Pure DVE + tiny ACT,
    zero PSUM usage, so these interleave into attention without stalling
    the in-order PE stream.

    When every LN affine is identity (the staged problem: a=1, b=0),
    LN(LN(x)) folds exactly to (x - m) / (s*(1+eps) + eps^2): the inner
    LN's output has mean 0 and std s/(s+eps), so the outer divide just
    rescales the same centered row. One bn_stats + one tensor_scalar
    per tile instead of two full passes."""
    work = pools["work"]

    if C["identity_ln"]:
        if src_sb is not None:
            xt = src_sb
        else:
            xt = work.tile([P, D], F32, tag="x_ln", bufs=8,
                           name=f"xln{blk}_{n}")
            # first chunk rides the (empty) ACT HWDGE queue instead of
            # queueing behind the constant-weight DMAs on SP
            eng = nc.scalar if (blk == 0 and n < 8) else nc.sync
            eng.dma_start(out=xt[:, :], in_=_rows(x_src_d, n * P, P))
        mv = work.tile([P, 6 + 2], F32, tag="ln_mv", bufs=8,
                       name=f"mvf_{blk}_{n}")
        nc.vector.bn_stats(out=mv[:, 0:6], in_=xt[:, :])
        nc.vector.bn_aggr(out=mv[:, 6:8], in_=mv[:, 0:6])
        r = work.tile([P, 1], F32, tag="ln_r", bufs=8,
                      name=f"rf_{blk}_{n}")
        nc.scalar.activation(out=r[:, :], in_=mv[:, 7:8], func=AF.Sqrt,
                             scale=float(D) / float(D - 1))
        nc.vector.tensor_scalar(out=r[:, :], in0=r[:, :],
                                scalar1=1.0 + EPS, scalar2=EPS * EPS,
                                op0=ALU.mult, op1=ALU.add)
        nc.vector.reciprocal(out=r[:, :], in_=r[:, :])
        zb = work.tile([P, D], BF16, tag=f"zb{blk}",
                       bufs=(NOT if blk == 1 else 8),
                       name=f"zb_{blk}_{n}")
        nc.vector.tensor_scalar(
            out=zb[:, :], in0=xt[:, :],
            scalar1=mv[:, 6:7], scalar2=r[:, 0:1],
            op0=ALU.subtract, op1=ALU.mult)
        return zb

    def _ln_pass(src_ap, m_ra, m_rb, dst_ap, uid):
        mv = work.tile([P, 6 + 2], F32, tag="ln_mv", name=f"mv_{uid}")
        nc.vector.bn_stats(out=mv[:, 0:6], in_=src_ap)
        nc.vector.bn_aggr(out=mv[:, 6:8], in_=mv[:, 0:6])
        r = work.tile([P, 1], F32, tag="ln_r", name=f"r_{uid}")
        nc.scalar.activation(out=r[:, :], in_=mv[:, 7:8], func=AF.Ln,
                             scale=float(D) / float(D - 1))
        nc.scalar.activation(out=r[:, :], in_=r[:, :], func=AF.Exp,
                             scale=0.5)
        nc.vector.tensor_scalar_add(out=r[:, :], in0=r[:, :], scalar1=EPS)
        nc.vector.reciprocal(out=r[:, :], in_=r[:, :])
        t = work.tile([P, D], F32, tag="ln_t", name=f"t_{uid}")
        nc.vector.tensor_scalar(
            out=t[:, :], in0=src_ap,
            scalar1=mv[:, 6:7], scalar2=r[:, 0:1],
            op0=ALU.subtract, op1=ALU.mult)
        nc.vector.tensor_mul(out=t[:, :], in0=t[:, :], in1=m_ra[:, :])
        nc.vector.tensor_add(out=dst_ap, in0=t[:, :], in1=m_rb[:, :])

    if src_sb is not None:
        xt = src_sb
    else:
        xt = work.tile([P, D], F32, tag="x_ln", name=f"xln{blk}_{n}")
        nc.sync.dma_start(out=xt[:, :], in_=_rows(x_src_d, n * P, P))
    yt = work.tile([P, D], F32, tag="y1", name=f"y1_{blk}_{n}")
    _ln_pass(xt[:, :], C["ra_bc"], C["rb_bc"], yt[:, :], f"{blk}_{n}a")
    zb = work.tile([P, D], BF16, tag=f"zb{blk}",
                   bufs=(NOT if blk == 1 else 4),
                   name=f"zb_{blk}_{n}")
    _ln_pass(yt[:, :], C["a0_bc"], C["b0_bc"], zb[:, :], f"{blk}_{n}b")
    return zb


def _transpose_tile(nc, pools, C, zb_ap, dst, col0, uid):
    """3 PE transposes of a [128, D] bf16 SBUF tile into dst[:, :, col0:..].
    PSUM->SBUF evacuation on ACT: it is idle in the LN valley / boundary,
    while DVE is the critical engine there. (XBAR DMA transposes are only
    used for the big DRAM gather halves: their 0-wait walrus limit turns
    every dependency into a queue-blocking NoOp, which serializes the LN
    pipeline when used per-tile.)"""
    psB = pools["psB"]
    for dt_ in range(DT):
        tp = psB.tile([P, P], BF16, tag="acc", name=f"tp{uid}_{dt_}")
        nc.tensor.transpose(out=tp[:, 0:P],
                            in_=zb_ap[:, dt_ * P:(dt_ + 1) * P],
                            identity=C["ident"][:, :])
        nc.vector.tensor_copy(out=dst[:, dt_, col0:col0 + P], in_=tp[:, 0:P])


# ---------------------------------------------------------------------------
def _build_block(nc, pools, C, x_src_d, out_d, blk,
                 zb_dram=None, premade_zb=None, after_qt=None,
                 out_sb=None, res_sb=None, mid_cb=None):
    """One residual MSA block, query-sharded to OWN rows.

    x_src_d: DRAM AP [*, D] f32 -- local rows (rotated order); rows
             0..OWN are this core's queries and residual base.
    out_d:   DRAM AP [OWN, D] f32 -- gets x_src[0:OWN] + MSA(...)[0:OWN]
             (ignored when out_sb is given).
    zb_dram: if set (block 2), two [S//2, D] bf16 DRAM halves with
             LN(LN(x2)) for the WHOLE sequence in gather order -- K/V
             source. If None (block 1), LN chains run locally.
    premade_zb: block 2 only -- the 8 locally-computed own zb tiles
             (rotated order), transposed into zTq for the Q projection.
    out_sb:  block 1 -- list that receives the 8 SBUF residual tiles
             instead of writing them to DRAM.
    res_sb:  block 2 -- those same tiles, used as the residual base.
    """
    work, psA, psB, psV, ste_pool, otp = (pools[k] for k in
                                          ("work", "psA", "psB", "psV",
                                           "ste", "ot"))

    qt_sb, kt_sb, v_aug = C["qt"], C["kt"], C["v_aug"]
    zTh = C["zTh"]          # two [P, DT, S//2] halves
    HNT = NTS // 2          # 16 token tiles per half

    def _q_proj(src):
        for dt_ in range(DT):
            for ntk in range(NQT):
                ps = psB.tile([P, QT], F32, tag="acc",
                              name=f"pq{blk}_{dt_}_{ntk}")
                for ki in range(DT):
                    nc.tensor.matmul(
                        ps[:, :],
                        lhsT=C["wqT"][:, ki, dt_ * P:(dt_ + 1) * P],
                        rhs=src[:, ki, ntk * QT:(ntk + 1) * QT],
                        start=(ki == 0), stop=(ki == DT - 1))
                nc.vector.tensor_scalar(
                    out=qt_sb[:, dt_, ntk * QT:(ntk + 1) * QT],
                    in0=ps[:, :], scalar1=C["bq_col"][:, dt_:dt_ + 1],
                    scalar2=None, op0=ALU.add)

    def _kv_z(n0, n1):
        """zT from local LN chains for token tiles n0..n1 (block 1).
        x rows land 4 tiles per DMA to cut HWDGE serialization."""
        for c in range(n0, n1, 4):
            xt4 = work.tile([P, 4, D], F32, tag="x_ln", bufs=3,
                            name=f"xt4_{blk}_{c}")
            eng = nc.scalar if c == 0 else nc.sync
            eng.dma_start(
                out=xt4[:, :, :],
                in_=bass.AP(tensor=x_src_d.tensor,
                            offset=x_src_d.offset + c * P * D,
                            ap=[[D, P], [P * D, 4], [1, D]]))
            for n in range(c, c + 4):
                zb = _emit_ln_tile(nc, pools, C, None, n, blk,
                                   src_sb=xt4[:, n - c, :])
                _transpose_tile(nc, pools, C, zb[:, :], zTh[n // HNT],
                                (n % HNT) * P, f"z{blk}_{n}")

    def _kv_z_half(h):
        """zT half h from the gathered DRAM buffer (block 2): one XBAR
        transpose; on the SP queue its wait-NoOps only delay later
        residual loads."""
        nc.sync.dma_start_transpose(zTh[h][:, :, :], zb_dram[h])

    def _kv_proj(n0, n1, act_evac=False):
        """K-proj + V_aug for token tiles n0..n1 (multiple of 4).

        act_evac routes the PSUM evacuations to ACT (zero-bias builds
        only) for phases where ACT idles and DVE is the constraint."""
        act_evac = act_evac and C["zero_bv"]
        for dt_ in range(DT):
            for ntk in range(n0 * P // QT, n1 * P // QT):
                zt_src = zTh[ntk * QT // (S // 2)]
                col = ntk * QT % (S // 2)
                ps = psB.tile([P, QT], F32, tag="acc",
                              name=f"pk{blk}_{dt_}_{ntk}")
                for ki in range(DT):
                    nc.tensor.matmul(
                        ps[:, :],
                        lhsT=C["wkT"][:, ki, dt_ * P:(dt_ + 1) * P],
                        rhs=zt_src[:, ki, col:col + QT],
                        start=(ki == 0), stop=(ki == DT - 1))
                if act_evac:
                    nc.scalar.copy(
                        out=kt_sb[:, dt_, ntk * QT:(ntk + 1) * QT],
                        in_=ps[:, :])
                else:
                    nc.vector.tensor_scalar(
                        out=kt_sb[:, dt_, ntk * QT:(ntk + 1) * QT],
                        in0=ps[:, :], scalar1=C["bk_col"][:, dt_:dt_ + 1],
                        scalar2=None, op0=ALU.add)
        for n in range(n0, n1):
            ps = psB.tile([P, QT], F32, tag="acc", name=f"v{blk}_{n}")
            for ki in range(DT):
                nc.tensor.matmul(
                    ps[:, :D],
                    lhsT=zTh[n // HNT][:, ki, (n % HNT) * P:(n % HNT + 1) * P],
                    rhs=C["wvT"][:, ki, :],
                    start=(ki == 0), stop=(ki == DT - 1))
            # ones-augmented layout; ones at j=DK persist from memset
            v_dst = v_aug[:, n, 0:VROW].rearrange(
                "p (h j) -> p h j", h=H, j=DK + 1)[:, :, 0:DK]
            v_in = ps[:, :D].rearrange("p (h j) -> p h j", h=H, j=DK)
            if act_evac:
                nc.scalar.copy(out=v_dst, in_=v_in)
            elif C["zero_bv"]:
                nc.vector.tensor_copy(out=v_dst, in_=v_in)
            else:
                nc.vector.tensor_tensor(
                    out=v_dst, in0=v_in,
                    in1=C["bv_bc"][:, :].rearrange("p (h j) -> p h j",
                                                   h=H, j=DK),
                    op=ALU.add)

    # ---- attention helpers (KCG=2: one fp8 DoubleRow pair per group) ----
    NG = NKC // KCG

    def _emit_group(ntk, hp, pv, g):
        kcs = list(range(g * KCG, (g + 1) * KCG))
        w = KCG * QT
        # both halves' score matmuls first (complementary tile_positions
        # on PE rows 0-63/64-127), then the exps, then the P*V consumers.
        st2, ste2 = [], []
        for half in range(2):       # head pair on partitions 0-63/64-127
            lo = half * DK
            st = psA.tile([P, KCG * QT], F32, tag="st",
                          name=f"st{blk}_{ntk}_{hp}_{g}_{half}")
            st2.append(st)
            for j, kc in enumerate(kcs):
                nc.tensor.matmul(
                    st[:, j * QT:(j + 1) * QT],
                    lhsT=kt_sb[lo:lo + DK, hp, kc * P:(kc + 1) * P],
                    rhs=qt_sb[lo:lo + DK, hp, ntk * QT:(ntk + 1) * QT],
                    start=True, stop=True)
        for half in range(2):
            ste = ste_pool.tile([P, KCG * QT], FP8, tag="ste",
                                name=f"se{blk}_{ntk}_{hp}_{g}_{half}")
            ste2.append(ste)
            nc.scalar.activation(out=ste[:, :w], in_=st2[half][:, :w],
                                 func=AF.Exp, scale=1.0 / 8.0)
        for half in range(2):
            h = 2 * hp + half
            kc = kcs[0]
            nc.tensor.matmul(
                pv[half][0:DK + 1, :],
                lhsT=v_aug[:, kc:kc + 2,
                           h * (DK + 1):(h + 1) * (DK + 1)],
                rhs=ste2[half][:, :].rearrange(
                    "p (k q) -> p k q", k=KCG, q=QT),
                start=(kc == 0), stop=(kc + 1 == NKC - 1),
                perf_mode=DR, skip_group_check=True)

    def _new_pv(ntk, hp):
        return [psV.tile([P, QT], F32, tag="pv",
                         name=f"pv{blk}_{ntk}_{hp}_{i}") for i in range(2)]

    def _hp_tail(ntk, hp, pv, ot):
        for half in range(2):
            lo = half * DK
            r_row = work.tile([1, QT], F32R, tag="r_row",
                              name=f"rr{blk}_{ntk}_{hp}_{half}")
            with nc.allow_low_precision(
                    reason="f32r broadcast of softmax denom"):
                nc.vector.reciprocal(
                    out=r_row[:, :], in_=pv[half][DK:DK + 1, :])
            # from psB (idle during the hp tails) so the st rotation isn't
            # blocked and the next hp's scores can start immediately
            r_bc = psB.tile([P, QT], F32, tag="acc",
                            name=f"rb{blk}_{ntk}_{hp}_{half}")
            nc.tensor.matmul(
                r_bc[0:DK, 0:QT],
                lhsT=C["ones"][0:1, 0:DK],
                rhs=r_row[0:1, :],
                start=True, stop=True)
            r_sb = work.tile([DK, QT], F32, tag="r_sb",
                             name=f"rs{blk}_{ntk}_{hp}_{half}")
            nc.vector.tensor_copy(out=r_sb[:, :], in_=r_bc[0:DK, 0:QT])
            nc.vector.tensor_tensor(
                out=ot[lo:lo + DK, hp, :],
                in0=pv[half][0:DK, :], in1=r_sb[:, :], op=ALU.mult)

    def _emit_out(ntk, ot):
        """Output projection + bias + residual for query tile ntk.

        Block 1 keeps the residual-stream tiles in SBUF (out_sb) -- no
        DRAM roundtrip; block 2 reads its residual base straight from
        those retained tiles and DMAs the final rows out.
        """
        for c4 in range(QT // P):
            tok = ntk * QT + c4 * P
            ps = psB.tile([P, QT], F32, tag="acc",
                          name=f"o{blk}_{ntk}_{c4}")
            for ki in range(DT):
                nc.tensor.matmul(
                    ps[:, :D],
                    lhsT=ot[:, ki, c4 * P:(c4 + 1) * P],
                    rhs=C["woT"][:, ki, :],
                    start=(ki == 0), stop=False)
            nc.tensor.matmul(
                ps[:, :D],
                lhsT=C["ones"][0:1, 0:P],
                rhs=C["bo_row"][0:1, :],
                start=False, stop=True, skip_group_check=True)
            if res_sb is not None:
                xr = res_sb[tok // P]
            else:
                xr = work.tile([P, D], F32, tag="x_res",
                               name=f"xr{blk}_{ntk}_{c4}")
                nc.sync.dma_start(out=xr[:, :], in_=_rows(x_src_d, tok, P))
            xo = work.tile([P, D], F32, tag=f"x_out{blk}",
                           bufs=(NOT if out_sb is not None else 3),
                           name=f"xo{blk}_{ntk}_{c4}")
            nc.vector.tensor_tensor(
                out=xo[:, :], in0=ps[:, :D], in1=xr[:, :], op=ALU.add)
            if out_sb is not None:
                out_sb[tok // P] = xo
            else:
                nc.sync.dma_start(out=_rows(out_d, tok, P), in_=xo[:, :])
        if after_qt is not None:
            after_qt(ntk)

    # ---- build: K/V interleaved with the first attention pass, so ACT
    # gets exp work while the front/boundary phases run ----
    ot0 = otp.tile([P, DT, QT], BF16, tag="ot", name=f"ot{blk}_0")
    pv0 = _new_pv(0, 0)
    if premade_zb is None:
        # block 1: all LN local; zTh[0] cols 0..OWN are exactly the own
        # queries. After 8 tiles the Q projection runs, then attention
        # groups (ntk=0, hp=0) chase the K/V chunks tile-availability.
        # Projections lag the LN stream one chunk so DVE's in-order queue
        # never parks a PSUM evacuation in front of the next chunk's LN.
        NCH = NTS // 4
        _kv_z(0, 4)
        _kv_z(4, 8), _kv_proj(0, 4)
        _q_proj(zTh[0])
        g_done = 0
        for c in range(2, NCH + 1):
            if c < NCH:
                _kv_z(4 * c, 4 * c + 4)
            _kv_proj(4 * (c - 1), 4 * c)
            g_av = min(2 * c, NG)
            for g in range(g_done, g_av):
                _emit_group(0, 0, pv0, g)
            g_done = g_av
    elif True:
        # block 2: gather0-dependent K/V first (ready the moment block 1
        # ends); the Q path was already emitted inside block 1 via the
        # after_qt callbacks (its ntk-0 half lands mid-block-1). While
        # gather1 is in flight, ALL THREE head-pairs process the half-0
        # keys -- partial pv accumulators spill to SBUF between head-pairs
        # so two PSUM banks suffice -- then reload and finish once half-1
        # lands.
        NGH = NG // 2
        _kv_z_half(0)
        for c in range(0, 4):
            _kv_proj(4 * c, 4 * c + 4)
        spills = []
        for hp in range(DT):
            pv = pv0 if hp == 0 else _new_pv(0, hp)
            for g in range(NGH):
                _emit_group(0, hp, pv, g)
            sb = work.tile([P, 2, QT], F32, tag="pvsp", bufs=3,
                           name=f"pvsp{hp}")
            for half in range(2):
                nc.vector.tensor_copy(out=sb[0:DK + 1, half, :],
                                      in_=pv[half][0:DK + 1, :])
            spills.append(sb)
        # psB fence: the Tile scheduler models the collective as ~free, so
        # without a data dependency it sprinkles the gather1-gated
        # projection matmuls into the PE queue AHEAD of the ready
        # spill-phase attention, head-blocking the engine for the whole
        # gather. Route both psB slots through dummies that depend on the
        # last head-pair's spill so those matmuls cannot be hoisted.
        fsc = work.tile([P, 2], F32, tag="fsc", name="fsc")
        for i in range(2):
            fce = psB.tile([P, QT], F32, tag="acc", name=f"fence{i}")
            nc.vector.tensor_copy(out=fce[0:1, 0:1],
                                  in_=spills[2][0:1, 1, 0:1])
            nc.vector.tensor_copy(out=fsc[0:1, i:i + 1], in_=fce[0:1, 0:1])
        if mid_cb is not None:
            mid_cb()
        _kv_z_half(1)
        # head-pair 0 reloads immediately and its groups chase the
        # projection chunks' tile availability, so ACT gets exp work
        # while DVE/PE grind through the half-1 K/V build
        pv = _new_pv(0, 0)
        for half in range(2):
            nc.vector.tensor_copy(out=pv[half][0:DK + 1, :],
                                  in_=spills[0][0:DK + 1, half, :])
        g_done = NGH
        for c in range(4, NTS // 4):
            _kv_proj(4 * c, 4 * c + 4)
            g_av = min(2 * c + 2, NG)
            for g in range(g_done, g_av):
                _emit_group(0, 0, pv, g)
            g_done = g_av
        _hp_tail(0, 0, pv, ot0)
        for hp in range(1, DT):
            pv = _new_pv(0, hp)
            for half in range(2):
                nc.vector.tensor_copy(out=pv[half][0:DK + 1, :],
                                      in_=spills[hp][0:DK + 1, half, :])
            for g in range(NGH, NG):
                _emit_group(0, hp, pv, g)
            _hp_tail(0, hp, pv, ot0)
        _emit_out(0, ot0)
    if premade_zb is None:
        _hp_tail(0, 0, pv0, ot0)
        for hp in range(1, DT):
            pv = _new_pv(0, hp)
            for g in range(NG):
                _emit_group(0, hp, pv, g)
            _hp_tail(0, hp, pv, ot0)
        _emit_out(0, ot0)
    for ntk in range(1, NQT):
        ot = otp.tile([P, DT, QT], BF16, tag="ot", name=f"ot{blk}_{ntk}")
        for hp in range(DT):
            pv = _new_pv(ntk, hp)
            for g in range(NG):
                _emit_group(ntk, hp, pv, g)
            _hp_tail(ntk, hp, pv, ot)
        _emit_out(ntk, ot)


def _build_program(identity_ln, zero_bv=True):
    nc = bass.Bass("TRN2", target_bir_lowering=False, debug=False,
                   num_devices=8)

    fast = identity_ln and zero_bv
    di = {}
    di["xs"] = nc.dram_tensor("xs", [S, D], F32, kind="ExternalInput")
    if fast:
        # packed weights + packed bias columns: 2 HWDGE slots at startup
        # instead of ~12 (the ramp is HWDGE-serialized)
        di["wAll"] = nc.dram_tensor("wAll", [4 * D, D], BF16,
                                    kind="ExternalInput")
        di["bqk_col"] = nc.dram_tensor("bqk_col", [P, 2 * DT], F32,
                                       kind="ExternalInput")
    else:
        for w in ("wqT", "wkT", "wvT", "woT"):
            di[w] = nc.dram_tensor(w, [D, D], BF16, kind="ExternalInput")
        di["bq_col"] = nc.dram_tensor("bq_col", [P, DT], F32,
                                      kind="ExternalInput")
        di["bk_col"] = nc.dram_tensor("bk_col", [P, DT], F32,
                                      kind="ExternalInput")
        di["bv_bc"] = nc.dram_tensor("bv_bc", [P, D], F32,
                                     kind="ExternalInput")
        for w in ("ra0_bc", "rb0_bc", "ra1_bc", "rb1_bc",
                  "a0_bc", "b0_bc"):
            di[w] = nc.dram_tensor(w, [P, D], F32, kind="ExternalInput")
    di["bo_row"] = nc.dram_tensor("bo_row", [1, D], F32R, kind="ExternalInput")
    di["ones_in"] = nc.dram_tensor("ones_in", [1, P], F32R,
                                   kind="ExternalInput")
    di["ident_in"] = nc.dram_tensor("ident_in", [P, P], BF16,
                                    kind="ExternalInput")
    out_d = nc.dram_tensor("out", [OWN, D], F32, kind="ExternalOutput")
    # split gather: half h carries each member's own rows [h*512:(h+1)*512];
    # separate tensors so first-half consumers never falsely depend on the
    # second collective.
    gin_d = [nc.dram_tensor(f"gin{h}", [OWN // 2, D], BF16) for h in range(2)]
    gout_d = [nc.dram_tensor(f"gout{h}", [S // 2, D], BF16) for h in range(2)]

    with tile.TileContext(nc) as tc:
        with tc.tile_pool(name="const", bufs=1) as const, \
             tc.tile_pool(name="work", bufs=3) as work, \
             tc.tile_pool(name="ot", bufs=2) as otp, \
             tc.tile_pool(name="ste", bufs=6) as ste_pool, \
             tc.tile_pool(name="psA", bufs=2, space="PSUM") as psA, \
             tc.tile_pool(name="psB", bufs=2, space="PSUM") as psB, \
             tc.tile_pool(name="psV", bufs=2, space="PSUM") as psV:

            pools = {"work": work, "psA": psA, "psB": psB, "psV": psV,
                     "ste": ste_pool, "ot": otp}

            C = {}
            if fast:
                wall = const.tile([P, 4, DT, D], BF16, name="wall")
                nc.sync.dma_start(
                    out=wall[:, :, :, :],
                    in_=di["wAll"][:].rearrange("(w d p) e -> p w d e",
                                                w=4, p=P))
                for i, wname in enumerate(("wqT", "wkT", "wvT", "woT")):
                    C[wname] = wall[:, i, :, :]
                bqk = const.tile([P, 2 * DT], F32, name="bqk")
                nc.sync.dma_start(out=bqk[:, :], in_=di["bqk_col"][:])
                C["bq_col"] = bqk[:, 0:DT]
                C["bk_col"] = bqk[:, DT:2 * DT]
            else:
                for wname in ("wqT", "wkT", "wvT", "woT"):
                    C[wname] = const.tile([P, DT, D], BF16, name=wname)
                    nc.sync.dma_start(
                        out=C[wname][:, :, :],
                        in_=di[wname][:].rearrange("(d p) e -> p d e", p=P))
                for wname in ("bq_col", "bk_col", "bv_bc"):
                    C[wname] = const.tile(list(di[wname].shape), F32,
                                          name=wname)
                    nc.sync.dma_start(out=C[wname][:], in_=di[wname][:])
                for wname in ("ra0_bc", "rb0_bc", "ra1_bc", "rb1_bc",
                              "a0_bc", "b0_bc"):
                    C[wname] = const.tile([P, D], F32, name=wname)
                    nc.sync.dma_start(out=C[wname][:, :], in_=di[wname][:])
            C["bo_row"] = const.tile([1, D], F32R, name="bo_row")
            nc.sync.dma_start(out=C["bo_row"][:], in_=di["bo_row"][:])
            C["ones"] = const.tile([1, P], F32R, name="ones")
            nc.sync.dma_start(out=C["ones"][:, :], in_=di["ones_in"][:])
            C["ident"] = const.tile([P, P], BF16, name="ident")
            nc.sync.dma_start(out=C["ident"][:, :], in_=di["ident_in"][:])

            C["zTh"] = [const.tile([P, DT, S // 2], BF16, name=f"zT{h}")
                        for h in range(2)]
            C["zTq"] = const.tile([P, DT, OWN], BF16, name="zTq")
            C["qt"] = const.tile([P, DT, OWN], BF16, name="qt")
            C["kt"] = const.tile([P, DT, S], BF16, name="kt")
            C["v_aug"] = const.tile([P, NTS, VROW2], FP8, name="v_aug")
            # only the per-head ones-columns (j=DK) need initializing --
            # data columns are overwritten by the V evacuations and the
            # pad tail is never read. A full memset is 12800 elem/lane on
            # DVE (~13us) parked at the head of the queue.
            nc.vector.memset(
                C["v_aug"][:, :, 0:VROW].rearrange(
                    "p n (h j) -> p n h j", h=H, j=DK + 1)[:, :, :, DK:DK + 1],
                1.0)

            C["identity_ln"] = identity_ln
            C["zero_bv"] = zero_bv
            C1 = dict(C)
            C2 = dict(C)
            if not fast:
                C1["ra_bc"], C1["rb_bc"] = C["ra0_bc"], C["rb0_bc"]
                C2["ra_bc"], C2["rb_bc"] = C["ra1_bc"], C["rb1_bc"]

            # block-2 LN of OWN rows interleaves into block-1 attention:
            # as each query tile's x2 rows land (in SBUF -- no DRAM
            # roundtrip), run the pure-DVE LN chain, stage the bf16 result
            # into that half's gather input, and kick the half-AllGather
            # that publishes it group-wide. The first gather runs
            # concurrently with block-1's second query tile.
            zb2 = [None] * NOT
            x2_sb = [None] * NOT

            def _b2_q(ntk):
                """Block-2 Q path for query tile ntk: zTq transposes of the
                own zb2 tiles plus the Q projection. Emitted from inside
                block 1 so the ntk-0 half is ready mid-block-1 and block-2
                attention can start the instant block 1 drains."""
                for n in range(ntk * 4, ntk * 4 + 4):
                    _transpose_tile(nc, pools, C, zb2[n][:, :], C["zTq"],
                                    n * P, f"q1_{n}")
                for dt_ in range(DT):
                    ps = psB.tile([P, QT], F32, tag="acc",
                                  name=f"pq1_{dt_}_{ntk}")
                    for ki in range(DT):
                        nc.tensor.matmul(
                            ps[:, :],
                            lhsT=C["wqT"][:, ki, dt_ * P:(dt_ + 1) * P],
                            rhs=C["zTq"][:, ki, ntk * QT:(ntk + 1) * QT],
                            start=(ki == 0), stop=(ki == DT - 1))
                    nc.vector.tensor_scalar(
                        out=C["qt"][:, dt_, ntk * QT:(ntk + 1) * QT],
                        in0=ps[:, :], scalar1=C["bq_col"][:, dt_:dt_ + 1],
                        scalar2=None, op0=ALU.add)

            def _b2_ln(ntk):
                for n in range(ntk * 4, ntk * 4 + 4):
                    zb2[n] = _emit_ln_tile(nc, pools, C2, None, n, 1,
                                           src_sb=x2_sb[n])
                    nc.sync.dma_start(
                        out=_rows(gin_d[ntk][:], (n % 4) * P, P),
                        in_=zb2[n][:, :])
                nc.gpsimd.collective_compute(
                    "AllGather", mybir.AluOpType.bypass,
                    replica_groups=CC_GROUPS,
                    ins=[gin_d[ntk][:].opt()], outs=[gout_d[ntk][:].opt()])
                if ntk == 0:
                    # ntk-1's Q path is deferred into block 2 (mid_cb):
                    # emitted here it would head-block the spill-phase
                    # attention behind its end-of-block-1 LN dependency
                    _b2_q(ntk)

            _build_block(nc, pools, C1, di["xs"][:], None, 0,
                         after_qt=_b2_ln, out_sb=x2_sb)
            _build_block(nc, pools, C2, None, out_d[:], 1,
                         zb_dram=[g[:] for g in gout_d], premade_zb=zb2,
                         res_sb=x2_sb, mid_cb=lambda: _b2_q(1))

    _fix_sync_waits(nc)
    return nc


_NC_CACHE = {}


def _get_nc(identity_ln=True, zero_bv=True):
    key = (identity_ln, zero_bv)
    if key not in _NC_CACHE:
        _NC_CACHE[key] = _build_program(identity_ln, zero_bv)
    return _NC_CACHE[key]


def _prep_inputs(x, a0, b0, ra0, rb0, ra1, rb1,
                 wq, bq, wk, bk, wv, bv, wo, bo):
    bf = ml_dtypes.bfloat16
    base = {
        "wqT": np.ascontiguousarray(np.asarray(wq, np.float32).T).astype(bf),
        "wkT": np.ascontiguousarray(np.asarray(wk, np.float32).T).astype(bf),
        "wvT": np.ascontiguousarray(np.asarray(wv, np.float32).T).astype(bf),
        "woT": np.ascontiguousarray(np.asarray(wo, np.float32).T).astype(bf),
        "bq_col": np.ascontiguousarray(
            np.asarray(bq, np.float32).reshape(DT, P).T),
        "bk_col": np.ascontiguousarray(
            np.asarray(bk, np.float32).reshape(DT, P).T),
        "bv_bc": np.ascontiguousarray(
            np.broadcast_to(np.asarray(bv, np.float32), (P, D))),
        "bo_row": np.asarray(bo, np.float32).reshape(1, D).copy(),
        "ra0_bc": np.ascontiguousarray(
            np.broadcast_to(np.asarray(ra0, np.float32), (P, D))),
        "rb0_bc": np.ascontiguousarray(
            np.broadcast_to(np.asarray(rb0, np.float32), (P, D))),
        "ra1_bc": np.ascontiguousarray(
            np.broadcast_to(np.asarray(ra1, np.float32), (P, D))),
        "rb1_bc": np.ascontiguousarray(
            np.broadcast_to(np.asarray(rb1, np.float32), (P, D))),
        "a0_bc": np.ascontiguousarray(
            np.broadcast_to(np.asarray(a0, np.float32), (P, D))),
        "b0_bc": np.ascontiguousarray(
            np.broadcast_to(np.asarray(b0, np.float32), (P, D))),
        "ones_in": np.ones((1, P), np.float32),
        "ident_in": np.eye(P, dtype=np.float32).astype(bf),
    }
    base["wAll"] = np.ascontiguousarray(np.concatenate(
        [base["wqT"], base["wkT"], base["wvT"], base["woT"]], axis=0))
    base["bqk_col"] = np.ascontiguousarray(
        np.concatenate([base["bq_col"], base["bk_col"]], axis=1))
    x = np.asarray(x, np.float32)
    in_maps = []
    for c in range(8):
        b, q0 = c // GRP, (c % GRP) * OWN
        m = dict(base)
        # rotate tokens so this core's output shard sits at rows 0..OWN
        m["xs"] = np.ascontiguousarray(
            np.concatenate([x[b, q0:], x[b, :q0]], axis=0))
        in_maps.append(m)
    return in_maps


def kernel(**inputs):
    identity_ln = all(
        bool(np.all(np.asarray(inputs[k], np.float32) == v))
        for k, v in (("a0", 1.0), ("b0", 0.0), ("ra0", 1.0), ("rb0", 0.0),
                     ("ra1", 1.0), ("rb1", 0.0)))
    zero_bv = all(
        bool(np.all(np.asarray(inputs[k], np.float32) == 0.0))
        for k in ("bq", "bk", "bv"))
    nc = _get_nc(identity_ln, zero_bv)
    in_maps = _prep_inputs(**inputs)
    res = run_bass_kernel_spmd(nc, in_maps, list(range(8)))
    B = inputs["x"].shape[0]
    out = np.empty((B, S, D), np.float32)
    for c in range(8):
        b, q0 = c // GRP, (c % GRP) * OWN
        out[b, q0:q0 + OWN, :] = res.results[c]["out"]
    return out


if __name__ == "__main__":
    rng = np.random.default_rng(0)
    ins = {
        "x": rng.standard_normal((2, S, D)).astype(np.float32),
        "a0": np.ones(D, np.float32), "b0": np.zeros(D, np.float32),
        "ra0": np.ones(D, np.float32), "rb0": np.zeros(D, np.float32),
        "ra1": np.ones(D, np.float32), "rb1": np.zeros(D, np.float32),
        "wq": (rng.standard_normal((D, D)) * 0.02).astype(np.float32),
        "bq": np.zeros(D, np.float32),
        "wk": (rng.standard_normal((D, D)) * 0.02).astype(np.float32),
        "bk": np.zeros(D, np.float32),
        "wv": (rng.standard_normal((D, D)) * 0.02).astype(np.float32),
        "bv": np.zeros(D, np.float32),
        "wo": (rng.standard_normal((D, D)) * 0.02).astype(np.float32),
        "bo": np.zeros(D, np.float32),
    }
    out = kernel(**ins)
    print("kernel ran, out shape", out.shape, out.dtype)


# revision 93
# speedup vs baseline: 1.2149x; 1.1658x over previous
"""Trainium2 Bass kernel for the nn_EncoderBlock problem.

Full inputs in, full output out. 8-way SPMD: cores 0-3 handle batch 0,
cores 4-7 batch 1. Within each 4-core batch group, BOTH blocks are
query-sharded 4 ways (1024 owned rows per core). Between the blocks, the
post-LayerNorm activations zb2 = LN(LN(x2)) (bf16, the exact values the
matmuls would consume anyway) are exchanged with a DRAM AllGather over
replica groups [[0..3],[4..7]] so every core can build block-2 K/V for
the full sequence. Block-2 keys are consumed in global row order (the
gather's order); attention is permutation-invariant over keys, so this
coexists with each core's rotated local query order.

All 8 cores run the SAME program: the host rotates each core's token
order by its query offset -- "queries 0..1023" on the device are exactly
the core's own output shard, while the key set stays complete.

Per block: LN(LN(x)) -> QKV projections -> per-head attention with
scores kept transposed [keys, queries] so softmax's exp doubles as the
PSUM->SBUF evacuation on the scalar engine (no max-subtraction needed:
|scores| < 2), P*V via a ones-augmented V (M=65) so the softmax
denominator falls out of the same matmul, normalization via a K=1
outer-product broadcast matmul, output projection with bias folded in
as a K=1 matmul, residual add. bf16 matmul operands, f32 accumulation,
f32 residual stream. Score matmuls are row-packed two heads at a time
(K=64 pairs on array rows 0-63/64-127).
"""

import sys

sys.path.insert(0, "/opt/trn_rl_repo")

import numpy as np
import ml_dtypes

import bass_rust
import concourse.bass as bass
import concourse.tile as tile
from concourse import mybir
from concourse.bass_utils import run_bass_kernel_spmd

F32 = mybir.dt.float32
F32R = mybir.dt.float32r
BF16 = mybir.dt.bfloat16
FP8 = mybir.dt.float8e4
AF = mybir.ActivationFunctionType
ALU = mybir.AluOpType
DR = mybir.MatmulPerfMode.DoubleRow

P = 128
D = 384
H = 6
DK = 64
DT = D // P          # 3 D-chunks of 128
S = 4096             # full sequence per batch
NTS = S // P         # 32 token tiles of 128
NKC = S // P         # 32 key chunks of 128
OWN = 1024           # query tokens owned per core (both blocks)
NOT = OWN // P       # 8 owned token tiles
GRP = 4              # cores per batch group
EPS = 1e-6
QT = 512             # query tile (free dim of score matmuls)
NQT = OWN // QT      # 2 query tiles per block
KCG = 2              # key chunks per exp group (one fp8 DoubleRow pair);
                     # st shrinks to 2 PSUM banks so the pv accumulators
                     # get their own pool and attention can interleave
                     # into the K/V build phase
VROW = H * (DK + 1)  # 390: per-kc row of V_aug (64 data cols + ones col/head)
VROW2 = 400          # padded to a 16-byte-multiple stride for DoubleRow
CC_GROUPS = [[0, 1, 2, 3], [4, 5, 6, 7]]

# ---------------------------------------------------------------------------
# walrus in this container caps sync-waits per instruction (1 for most,
# 0 for DMA-transpose). Hoist excess waits onto same-engine NoOps.
_WAIT_LIMIT_BY_TYPE = {"InstDmaTransposeAnt": 0}
_wfix_ctr = [0]


def _fix_sync_waits(nc):
    for f in nc.m.functions:
        for bb in f.blocks:
            out = []
            changed = False
            for ins in bb.instructions:
                si = ins.sync_info
                waits = list(si.on_wait) if si is not None else []
                limit = _WAIT_LIMIT_BY_TYPE.get(type(ins).__name__, 1)
                if len(waits) > limit:
                    keep, hoist = waits[:limit], waits[limit:]
                    for w in hoist:
                        _wfix_ctr[0] += 1
                        nop = mybir.InstNoOp(
                            name=f"WFIX-{_wfix_ctr[0]}", engine=ins.engine
                        )
                        nop.sync_info = bass_rust.SyncInfo(on_wait=[w], on_update=[])
                        out.append(nop)
                    ins.sync_info = bass_rust.SyncInfo(
                        on_wait=keep, on_update=list(si.on_update)
                    )
                    changed = True
                out.append(ins)
            if changed:
                bb.instructions = out


def _rows(dram_ap, row0, nrows):
    """[nrows, D] rows of a [*, D] DRAM tensor as a DMA AP."""
    return bass.AP(tensor=dram_ap.tensor,
                   offset=dram_ap.offset + row0 * D,
                   ap=[[D, nrows], [1, D]])


# ---------------------------------------------------------------------------
def _emit_ln_tile(nc, pools, C, x_src_d, n, blk, src_sb=None):
    """LN(LN(x)) for one 128-token tile -> zb (bf16). Pure DVE + tiny ACT,
    zero PSUM usage, so these interleave into attention without stalling
    the in-order PE stream.

    When every LN affine is identity (the staged problem: a=1, b=0),
    LN(LN(x)) folds exactly to (x - m) / (s*(1+eps) + eps^2): the inner
    LN's output has mean 0 and std s/(s+eps), so the outer divide just
    rescales the same centered row. One bn_stats + one tensor_scalar
    per tile instead of two full passes."""
    work = pools["work"]

    if C["identity_ln"]:
        if src_sb is not None:
            xt = src_sb
        else:
            xt = work.tile([P, D], F32, tag="x_ln", bufs=8,
                           name=f"xln{blk}_{n}")
            # first chunk rides the (empty) ACT HWDGE queue instead of
            # queueing behind the constant-weight DMAs on SP
            eng = nc.scalar if (blk == 0 and n < 8) else nc.sync
            eng.dma_start(out=xt[:, :], in_=_rows(x_src_d, n * P, P))
        mv = work.tile([P, 6 + 2], F32, tag="ln_mv", bufs=8,
                       name=f"mvf_{blk}_{n}")
        nc.vector.bn_stats(out=mv[:, 0:6], in_=xt[:, :])
        nc.vector.bn_aggr(out=mv[:, 6:8], in_=mv[:, 0:6])
        r = work.tile([P, 1], F32, tag="ln_r", bufs=8,
                      name=f"rf_{blk}_{n}")
        nc.scalar.activation(out=r[:, :], in_=mv[:, 7:8], func=AF.Sqrt,
                             scale=float(D) / float(D - 1))
        nc.vector.tensor_scalar(out=r[:, :], in0=r[:, :],
                                scalar1=1.0 + EPS, scalar2=EPS * EPS,
                                op0=ALU.mult, op1=ALU.add)
        nc.vector.reciprocal(out=r[:, :], in_=r[:, :])
        zb = work.tile([P, D], BF16, tag=f"zb{blk}",
                       bufs=(NOT if blk == 1 else 8),
                       name=f"zb_{blk}_{n}")
        nc.vector.tensor_scalar(
            out=zb[:, :], in0=xt[:, :],
            scalar1=mv[:, 6:7], scalar2=r[:, 0:1],
            op0=ALU.subtract, op1=ALU.mult)
        return zb

    def _ln_pass(src_ap, m_ra, m_rb, dst_ap, uid):
        mv = work.tile([P, 6 + 2], F32, tag="ln_mv", name=f"mv_{uid}")
        nc.vector.bn_stats(out=mv[:, 0:6], in_=src_ap)
        nc.vector.bn_aggr(out=mv[:, 6:8], in_=mv[:, 0:6])
        r = work.tile([P, 1], F32, tag="ln_r", name=f"r_{uid}")
        nc.scalar.activation(out=r[:, :], in_=mv[:, 7:8], func=AF.Ln,
                             scale=float(D) / float(D - 1))
        nc.scalar.activation(out=r[:, :], in_=r[:, :], func=AF.Exp,
                             scale=0.5)
        nc.vector.tensor_scalar_add(out=r[:, :], in0=r[:, :], scalar1=EPS)
        nc.vector.reciprocal(out=r[:, :], in_=r[:, :])
        t = work.tile([P, D], F32, tag="ln_t", name=f"t_{uid}")
        nc.vector.tensor_scalar(
            out=t[:, :], in0=src_ap,
            scalar1=mv[:, 6:7], scalar2=r[:, 0:1],
            op0=ALU.subtract, op1=ALU.mult)
        nc.vector.tensor_mul(out=t[:, :], in0=t[:, :], in1=m_ra[:, :])
        nc.vector.tensor_add(out=dst_ap, in0=t[:, :], in1=m_rb[:, :])

    if src_sb is not None:
        xt = src_sb
    else:
        xt = work.tile([P, D], F32, tag="x_ln", name=f"xln{blk}_{n}")
        nc.sync.dma_start(out=xt[:, :], in_=_rows(x_src_d, n * P, P))
    yt = work.tile([P, D], F32, tag="y1", name=f"y1_{blk}_{n}")
    _ln_pass(xt[:, :], C["ra_bc"], C["rb_bc"], yt[:, :], f"{blk}_{n}a")
    zb = work.tile([P, D], BF16, tag=f"zb{blk}",
                   bufs=(NOT if blk == 1 else 4),
                   name=f"zb_{blk}_{n}")
    _ln_pass(yt[:, :], C["a0_bc"], C["b0_bc"], zb[:, :], f"{blk}_{n}b")
    return zb


def _transpose_tile(nc, pools, C, zb_ap, dst, col0, uid):
    """3 PE transposes of a [128, D] bf16 SBUF tile into dst[:, :, col0:..].
    PSUM->SBUF evacuation on ACT: it is idle in the LN valley / boundary,
    while DVE is the critical engine there. (XBAR DMA transposes are only
    used for the big DRAM gather halves: their 0-wait walrus limit turns
    every dependency into a queue-blocking NoOp, which serializes the LN
    pipeline when used per-tile.)"""
    psB = pools["psB"]
    for dt_ in range(DT):
        tp = psB.tile([P, P], BF16, tag="acc", name=f"tp{uid}_{dt_}")
        nc.tensor.transpose(out=tp[:, 0:P],
                            in_=zb_ap[:, dt_ * P:(dt_ + 1) * P],
                            identity=C["ident"][:, :])
        nc.vector.tensor_copy(out=dst[:, dt_, col0:col0 + P], in_=tp[:, 0:P])


# ---------------------------------------------------------------------------
def _build_block(nc, pools, C, x_src_d, out_d, blk,
                 zb_dram=None, premade_zb=None, after_qt=None,
                 out_sb=None, res_sb=None, mid_cb=None):
    """One residual MSA block, query-sharded to OWN rows.

    x_src_d: DRAM AP [*, D] f32 -- local rows (rotated order); rows
             0..OWN are this core's queries and residual base.
    out_d:   DRAM AP [OWN, D] f32 -- gets x_src[0:OWN] + MSA(...)[0:OWN]
             (ignored when out_sb is given).
    zb_dram: if set (block 2), two [S//2, D] bf16 DRAM halves with
             LN(LN(x2)) for the WHOLE sequence in gather order -- K/V
             source. If None (block 1), LN chains run locally.
    premade_zb: block 2 only -- the 8 locally-computed own zb tiles
             (rotated order), transposed into zTq for the Q projection.
    out_sb:  block 1 -- list that receives the 8 SBUF residual tiles
             instead of writing them to DRAM.
    res_sb:  block 2 -- those same tiles, used as the residual base.
    """
    work, psA, psB, psV, ste_pool, otp = (pools[k] for k in
                                          ("work", "psA", "psB", "psV",
                                           "ste", "ot"))

    qt_sb, kt_sb, v_aug = C["qt"], C["kt"], C["v_aug"]
    zTh = C["zTh"]          # two [P, DT, S//2] halves
    HNT = NTS // 2          # 16 token tiles per half

    def _q_proj(src):
        for dt_ in range(DT):
            for ntk in range(NQT):
                ps = psB.tile([P, QT], F32, tag="acc",
                              name=f"pq{blk}_{dt_}_{ntk}")
                for ki in range(DT):
                    nc.tensor.matmul(
                        ps[:, :],
                        lhsT=C["wqT"][:, ki, dt_ * P:(dt_ + 1) * P],
                        rhs=src[:, ki, ntk * QT:(ntk + 1) * QT],
                        start=(ki == 0), stop=(ki == DT - 1))
                nc.vector.tensor_scalar(
                    out=qt_sb[:, dt_, ntk * QT:(ntk + 1) * QT],
                    in0=ps[:, :], scalar1=C["bq_col"][:, dt_:dt_ + 1],
                    scalar2=None, op0=ALU.add)

    def _kv_z(n0, n1):
        """zT from local LN chains for token tiles n0..n1 (block 1).
        x rows land 4 tiles per DMA to cut HWDGE serialization."""
        for c in range(n0, n1, 4):
            xt4 = work.tile([P, 4, D], F32, tag="x_ln", bufs=3,
                            name=f"xt4_{blk}_{c}")
            eng = nc.scalar if c == 0 else nc.sync
            eng.dma_start(
                out=xt4[:, :, :],
                in_=bass.AP(tensor=x_src_d.tensor,
                            offset=x_src_d.offset + c * P * D,
                            ap=[[D, P], [P * D, 4], [1, D]]))
            for n in range(c, c + 4):
                zb = _emit_ln_tile(nc, pools, C, None, n, blk,
                                   src_sb=xt4[:, n - c, :])
                _transpose_tile(nc, pools, C, zb[:, :], zTh[n // HNT],
                                (n % HNT) * P, f"z{blk}_{n}")

    def _kv_z_half(h):
        """zT half h from the gathered DRAM buffer (block 2): one XBAR
        transpose; on the SP queue its wait-NoOps only delay later
        residual loads."""
        nc.sync.dma_start_transpose(zTh[h][:, :, :], zb_dram[h])

    def _kv_proj(n0, n1, act_evac=False, v_act=False):
        """K-proj + V_aug for token tiles n0..n1 (multiple of 4).

        act_evac routes the PSUM evacuations to ACT (zero-bias builds
        only) for phases where ACT idles and DVE is the constraint."""
        act_evac = act_evac and C["zero_bv"]
        for dt_ in range(DT):
            for ntk in range(n0 * P // QT, n1 * P // QT):
                zt_src = zTh[ntk * QT // (S // 2)]
                col = ntk * QT % (S // 2)
                ps = psB.tile([P, QT], F32, tag="acc",
                              name=f"pk{blk}_{dt_}_{ntk}")
                for ki in range(DT):
                    nc.tensor.matmul(
                        ps[:, :],
                        lhsT=C["wkT"][:, ki, dt_ * P:(dt_ + 1) * P],
                        rhs=zt_src[:, ki, col:col + QT],
                        start=(ki == 0), stop=(ki == DT - 1))
                if act_evac:
                    nc.scalar.copy(
                        out=kt_sb[:, dt_, ntk * QT:(ntk + 1) * QT],
                        in_=ps[:, :])
                else:
                    nc.vector.tensor_scalar(
                        out=kt_sb[:, dt_, ntk * QT:(ntk + 1) * QT],
                        in0=ps[:, :], scalar1=C["bk_col"][:, dt_:dt_ + 1],
                        scalar2=None, op0=ALU.add)
        for n in range(n0, n1):
            ps = psB.tile([P, QT], F32, tag="acc", name=f"v{blk}_{n}")
            for ki in range(DT):
                nc.tensor.matmul(
                    ps[:, :D],
                    lhsT=zTh[n // HNT][:, ki, (n % HNT) * P:(n % HNT + 1) * P],
                    rhs=C["wvT"][:, ki, :],
                    start=(ki == 0), stop=(ki == DT - 1))
            # ones-augmented layout; ones at j=DK persist from memset
            v_dst = v_aug[:, n, 0:VROW].rearrange(
                "p (h j) -> p h j", h=H, j=DK + 1)[:, :, 0:DK]
            v_in = ps[:, :D].rearrange("p (h j) -> p h j", h=H, j=DK)
            if (act_evac or v_act) and C["zero_bv"]:
                nc.scalar.copy(out=v_dst, in_=v_in)
            elif C["zero_bv"]:
                nc.vector.tensor_copy(out=v_dst, in_=v_in)
            else:
                nc.vector.tensor_tensor(
                    out=v_dst, in0=v_in,
                    in1=C["bv_bc"][:, :].rearrange("p (h j) -> p h j",
                                                   h=H, j=DK),
                    op=ALU.add)

    # ---- attention helpers (KCG=2: one fp8 DoubleRow pair per group) ----
    NG = NKC // KCG

    def _emit_group(ntk, hp, pv, g):
        kcs = list(range(g * KCG, (g + 1) * KCG))
        w = KCG * QT
        # both halves' score matmuls first (complementary tile_positions
        # on PE rows 0-63/64-127), then the exps, then the P*V consumers.
        st2, ste2 = [], []
        for half in range(2):       # head pair on partitions 0-63/64-127
            lo = half * DK
            st = psA.tile([P, KCG * QT], F32, tag="st",
                          name=f"st{blk}_{ntk}_{hp}_{g}_{half}")
            st2.append(st)
            for j, kc in enumerate(kcs):
                nc.tensor.matmul(
                    st[:, j * QT:(j + 1) * QT],
                    lhsT=kt_sb[lo:lo + DK, hp, kc * P:(kc + 1) * P],
                    rhs=qt_sb[lo:lo + DK, hp, ntk * QT:(ntk + 1) * QT],
                    start=True, stop=True)
        for half in range(2):
            ste = ste_pool.tile([P, KCG * QT], FP8, tag="ste",
                                name=f"se{blk}_{ntk}_{hp}_{g}_{half}")
            ste2.append(ste)
            nc.scalar.activation(out=ste[:, :w], in_=st2[half][:, :w],
                                 func=AF.Exp, scale=1.0 / 8.0)
        for half in range(2):
            h = 2 * hp + half
            kc = kcs[0]
            nc.tensor.matmul(
                pv[half][0:DK + 1, :],
                lhsT=v_aug[:, kc:kc + 2,
                           h * (DK + 1):(h + 1) * (DK + 1)],
                rhs=ste2[half][:, :].rearrange(
                    "p (k q) -> p k q", k=KCG, q=QT),
                start=(kc == 0), stop=(kc + 1 == NKC - 1),
                perf_mode=DR, skip_group_check=True)

    def _new_pv(ntk, hp):
        return [psV.tile([P, QT], F32, tag="pv",
                         name=f"pv{blk}_{ntk}_{hp}_{i}") for i in range(2)]

    def _hp_tail(ntk, hp, pv, ot):
        for half in range(2):
            lo = half * DK
            r_row = work.tile([1, QT], F32R, tag="r_row",
                              name=f"rr{blk}_{ntk}_{hp}_{half}")
            with nc.allow_low_precision(
                    reason="f32r broadcast of softmax denom"):
                nc.vector.reciprocal(
                    out=r_row[:, :], in_=pv[half][DK:DK + 1, :])
            # from psB (idle during the hp tails) so the st rotation isn't
            # blocked and the next hp's scores can start immediately
            r_bc = psB.tile([P, QT], F32, tag="acc",
                            name=f"rb{blk}_{ntk}_{hp}_{half}")
            nc.tensor.matmul(
                r_bc[0:DK, 0:QT],
                lhsT=C["ones"][0:1, 0:DK],
                rhs=r_row[0:1, :],
                start=True, stop=True)
            r_sb = work.tile([DK, QT], F32, tag="r_sb",
                             name=f"rs{blk}_{ntk}_{hp}_{half}")
            nc.vector.tensor_copy(out=r_sb[:, :], in_=r_bc[0:DK, 0:QT])
            nc.vector.tensor_tensor(
                out=ot[lo:lo + DK, hp, :],
                in0=pv[half][0:DK, :], in1=r_sb[:, :], op=ALU.mult)

    def _emit_out(ntk, ot):
        """Output projection + bias + residual for query tile ntk.

        Block 1 keeps the residual-stream tiles in SBUF (out_sb) -- no
        DRAM roundtrip; block 2 reads its residual base straight from
        those retained tiles and DMAs the final rows out.
        """
        for c4 in range(QT // P):
            tok = ntk * QT + c4 * P
            ps = psB.tile([P, QT], F32, tag="acc",
                          name=f"o{blk}_{ntk}_{c4}")
            for ki in range(DT):
                nc.tensor.matmul(
                    ps[:, :D],
                    lhsT=ot[:, ki, c4 * P:(c4 + 1) * P],
                    rhs=C["woT"][:, ki, :],
                    start=(ki == 0), stop=False)
            nc.tensor.matmul(
                ps[:, :D],
                lhsT=C["ones"][0:1, 0:P],
                rhs=C["bo_row"][0:1, :],
                start=False, stop=True, skip_group_check=True)
            if res_sb is not None:
                xr = res_sb[tok // P]
            else:
                xr = work.tile([P, D], F32, tag="x_res",
                               name=f"xr{blk}_{ntk}_{c4}")
                nc.sync.dma_start(out=xr[:, :], in_=_rows(x_src_d, tok, P))
            xo = work.tile([P, D], F32, tag=f"x_out{blk}",
                           bufs=(NOT if out_sb is not None else 3),
                           name=f"xo{blk}_{ntk}_{c4}")
            nc.vector.tensor_tensor(
                out=xo[:, :], in0=ps[:, :D], in1=xr[:, :], op=ALU.add)
            if out_sb is not None:
                out_sb[tok // P] = xo
            else:
                nc.sync.dma_start(out=_rows(out_d, tok, P), in_=xo[:, :])
        if after_qt is not None:
            after_qt(ntk)

    # ---- build: K/V interleaved with the first attention pass, so ACT
    # gets exp work while the front/boundary phases run ----
    ot0 = otp.tile([P, DT, QT], BF16, tag="ot", name=f"ot{blk}_0")
    pv0 = _new_pv(0, 0)
    if premade_zb is None:
        # block 1: all LN local; zTh[0] cols 0..OWN are exactly the own
        # queries. After 8 tiles the Q projection runs, then attention
        # groups (ntk=0, hp=0) chase the K/V chunks tile-availability.
        # Projections lag the LN stream one chunk so DVE's in-order queue
        # never parks a PSUM evacuation in front of the next chunk's LN.
        NCH = NTS // 4
        _kv_z(0, 4)
        _kv_z(4, 8), _kv_proj(0, 4)
        _q_proj(zTh[0])
        g_done = 0
        for c in range(2, NCH + 1):
            if c < NCH:
                _kv_z(4 * c, 4 * c + 4)
            _kv_proj(4 * (c - 1), 4 * c)
            g_av = min(2 * c, NG)
            for g in range(g_done, g_av):
                _emit_group(0, 0, pv0, g)
            g_done = g_av
    elif True:
        # block 2: gather0-dependent K/V first (ready the moment block 1
        # ends); the Q path was already emitted inside block 1 via the
        # after_qt callbacks (its ntk-0 half lands mid-block-1). While
        # gather1 is in flight, ALL THREE head-pairs process the half-0
        # keys -- partial pv accumulators spill to SBUF between head-pairs
        # so two PSUM banks suffice -- then reload and finish once half-1
        # lands.
        NGH = NG // 2
        _kv_z_half(0)
        for c in range(0, 4):
            _kv_proj(4 * c, 4 * c + 4)
        spills = []

        def _spill(seg_ntk, seg_hp, pv):
            for g in range(NGH):
                _emit_group(seg_ntk, seg_hp, pv, g)
            sb = work.tile([P, 2, QT], F32, tag="pvsp", bufs=4,
                           name=f"pvsp{len(spills)}")
            for half in range(2):
                nc.vector.tensor_copy(out=sb[0:DK + 1, half, :],
                                      in_=pv[half][0:DK + 1, :])
            spills.append(sb)

        for hp in range(DT):
            _spill(0, hp, pv0 if hp == 0 else _new_pv(0, hp))
        # ntk-1's Q path (deps ready just after block 1) followed by its
        # hp-0 half-0 groups: together with the three segments above this
        # covers the whole gather1 window with exp work
        if mid_cb is not None:
            mid_cb()
        _spill(1, 0, _new_pv(1, 0))
        # psB fence: the Tile scheduler models the collective as ~free, so
        # without a data dependency it sprinkles the gather1-gated
        # projection matmuls into the PE queue AHEAD of the ready
        # spill-phase attention, head-blocking the engine for the whole
        # gather. Route both psB slots through dummies that depend on the
        # last segment's spill so those matmuls cannot be hoisted.
        fsc = work.tile([P, 2], F32, tag="fsc", name="fsc")
        for i in range(2):
            fce = psB.tile([P, QT], F32, tag="acc", name=f"fence{i}")
            nc.vector.tensor_copy(out=fce[0:1, 0:1],
                                  in_=spills[3][0:1, 1, 0:1])
            nc.vector.tensor_copy(out=fsc[0:1, i:i + 1], in_=fce[0:1, 0:1])
        _kv_z_half(1)
        # head-pair 0 reloads immediately and its groups chase the
        # projection chunks' tile availability, so ACT gets exp work
        # while DVE/PE grind through the half-1 K/V build
        pv = _new_pv(0, 0)
        for half in range(2):
            nc.vector.tensor_copy(out=pv[half][0:DK + 1, :],
                                  in_=spills[0][0:DK + 1, half, :])
        g_done = NGH
        for c in range(4, NTS // 4):
            _kv_proj(4 * c, 4 * c + 4)
            g_av = min(2 * c + 2, NG)
            for g in range(g_done, g_av):
                _emit_group(0, 0, pv, g)
            g_done = g_av
        _hp_tail(0, 0, pv, ot0)
        for hp in range(1, DT):
            pv = _new_pv(0, hp)
            for half in range(2):
                nc.vector.tensor_copy(out=pv[half][0:DK + 1, :],
                                      in_=spills[hp][0:DK + 1, half, :])
            for g in range(NGH, NG):
                _emit_group(0, hp, pv, g)
            _hp_tail(0, hp, pv, ot0)
        _emit_out(0, ot0)
        # ntk 1: hp-0 resumes from its spill, the rest run start-to-end
        ot1 = otp.tile([P, DT, QT], BF16, tag="ot", name=f"ot{blk}_1")
        for hp in range(DT):
            pv = _new_pv(1, hp)
            if hp == 0:
                for half in range(2):
                    nc.vector.tensor_copy(out=pv[half][0:DK + 1, :],
                                          in_=spills[3][0:DK + 1, half, :])
            for g in range(0 if hp else NGH, NG):
                _emit_group(1, hp, pv, g)
            _hp_tail(1, hp, pv, ot1)
        _emit_out(1, ot1)
    if premade_zb is None:
        _hp_tail(0, 0, pv0, ot0)
        for hp in range(1, DT):
            pv = _new_pv(0, hp)
            for g in range(NG):
                _emit_group(0, hp, pv, g)
            _hp_tail(0, hp, pv, ot0)
        _emit_out(0, ot0)
        for ntk in range(1, NQT):
            ot = otp.tile([P, DT, QT], BF16, tag="ot",
                          name=f"ot{blk}_{ntk}")
            for hp in range(DT):
                pv = _new_pv(ntk, hp)
                for g in range(NG):
                    _emit_group(ntk, hp, pv, g)
                _hp_tail(ntk, hp, pv, ot)
            _emit_out(ntk, ot)


def _build_program(identity_ln, zero_bv=True):
    nc = bass.Bass("TRN2", target_bir_lowering=False, debug=False,
                   num_devices=8)

    fast = identity_ln and zero_bv
    di = {}
    di["xs"] = nc.dram_tensor("xs", [S, D], F32, kind="ExternalInput")
    if fast:
        # packed weights + packed bias columns: 2 HWDGE slots at startup
        # instead of ~12 (the ramp is HWDGE-serialized)
        di["wAll"] = nc.dram_tensor("wAll", [4 * D, D], BF16,
                                    kind="ExternalInput")
        di["bqk_col"] = nc.dram_tensor("bqk_col", [P, 2 * DT], F32,
                                       kind="ExternalInput")
    else:
        for w in ("wqT", "wkT", "wvT", "woT"):
            di[w] = nc.dram_tensor(w, [D, D], BF16, kind="ExternalInput")
        di["bq_col"] = nc.dram_tensor("bq_col", [P, DT], F32,
                                      kind="ExternalInput")
        di["bk_col"] = nc.dram_tensor("bk_col", [P, DT], F32,
                                      kind="ExternalInput")
        di["bv_bc"] = nc.dram_tensor("bv_bc", [P, D], F32,
                                     kind="ExternalInput")
        for w in ("ra0_bc", "rb0_bc", "ra1_bc", "rb1_bc",
                  "a0_bc", "b0_bc"):
            di[w] = nc.dram_tensor(w, [P, D], F32, kind="ExternalInput")
    di["bo_row"] = nc.dram_tensor("bo_row", [1, D], F32R, kind="ExternalInput")
    di["ones_in"] = nc.dram_tensor("ones_in", [1, P], F32R,
                                   kind="ExternalInput")
    di["ident_in"] = nc.dram_tensor("ident_in", [P, P], BF16,
                                    kind="ExternalInput")
    out_d = nc.dram_tensor("out", [OWN, D], F32, kind="ExternalOutput")
    # split gather: half h carries each member's own rows [h*512:(h+1)*512];
    # separate tensors so first-half consumers never falsely depend on the
    # second collective.
    gin_d = [nc.dram_tensor(f"gin{h}", [OWN // 2, D], BF16) for h in range(2)]
    gout_d = [nc.dram_tensor(f"gout{h}", [S // 2, D], BF16) for h in range(2)]

    with tile.TileContext(nc) as tc:
        with tc.tile_pool(name="const", bufs=1) as const, \
             tc.tile_pool(name="work", bufs=3) as work, \
             tc.tile_pool(name="ot", bufs=2) as otp, \
             tc.tile_pool(name="ste", bufs=6) as ste_pool, \
             tc.tile_pool(name="psA", bufs=2, space="PSUM") as psA, \
             tc.tile_pool(name="psB", bufs=2, space="PSUM") as psB, \
             tc.tile_pool(name="psV", bufs=2, space="PSUM") as psV:

            pools = {"work": work, "psA": psA, "psB": psB, "psV": psV,
                     "ste": ste_pool, "ot": otp}

            C = {}
            if fast:
                wall = const.tile([P, 4, DT, D], BF16, name="wall")
                nc.sync.dma_start(
                    out=wall[:, :, :, :],
                    in_=di["wAll"][:].rearrange("(w d p) e -> p w d e",
                                                w=4, p=P))
                for i, wname in enumerate(("wqT", "wkT", "wvT", "woT")):
                    C[wname] = wall[:, i, :, :]
                bqk = const.tile([P, 2 * DT], F32, name="bqk")
                nc.sync.dma_start(out=bqk[:, :], in_=di["bqk_col"][:])
                C["bq_col"] = bqk[:, 0:DT]
                C["bk_col"] = bqk[:, DT:2 * DT]
            else:
                for wname in ("wqT", "wkT", "wvT", "woT"):
                    C[wname] = const.tile([P, DT, D], BF16, name=wname)
                    nc.sync.dma_start(
                        out=C[wname][:, :, :],
                        in_=di[wname][:].rearrange("(d p) e -> p d e", p=P))
                for wname in ("bq_col", "bk_col", "bv_bc"):
                    C[wname] = const.tile(list(di[wname].shape), F32,
                                          name=wname)
                    nc.sync.dma_start(out=C[wname][:], in_=di[wname][:])
                for wname in ("ra0_bc", "rb0_bc", "ra1_bc", "rb1_bc",
                              "a0_bc", "b0_bc"):
                    C[wname] = const.tile([P, D], F32, name=wname)
                    nc.sync.dma_start(out=C[wname][:, :], in_=di[wname][:])
            C["bo_row"] = const.tile([1, D], F32R, name="bo_row")
            nc.sync.dma_start(out=C["bo_row"][:], in_=di["bo_row"][:])
            C["ones"] = const.tile([1, P], F32R, name="ones")
            nc.sync.dma_start(out=C["ones"][:, :], in_=di["ones_in"][:])
            C["ident"] = const.tile([P, P], BF16, name="ident")
            nc.sync.dma_start(out=C["ident"][:, :], in_=di["ident_in"][:])

            C["zTh"] = [const.tile([P, DT, S // 2], BF16, name=f"zT{h}")
                        for h in range(2)]
            C["zTq"] = const.tile([P, DT, OWN], BF16, name="zTq")
            C["qt"] = const.tile([P, DT, OWN], BF16, name="qt")
            C["kt"] = const.tile([P, DT, S], BF16, name="kt")
            C["v_aug"] = const.tile([P, NTS, VROW2], FP8, name="v_aug")
            # only the per-head ones-columns (j=DK) need initializing --
            # data columns are overwritten by the V evacuations and the
            # pad tail is never read. A full memset is 12800 elem/lane on
            # DVE (~13us) parked at the head of the queue.
            nc.vector.memset(
                C["v_aug"][:, :, 0:VROW].rearrange(
                    "p n (h j) -> p n h j", h=H, j=DK + 1)[:, :, :, DK:DK + 1],
                1.0)

            C["identity_ln"] = identity_ln
            C["zero_bv"] = zero_bv
            C1 = dict(C)
            C2 = dict(C)
            if not fast:
                C1["ra_bc"], C1["rb_bc"] = C["ra0_bc"], C["rb0_bc"]
                C2["ra_bc"], C2["rb_bc"] = C["ra1_bc"], C["rb1_bc"]

            # block-2 LN of OWN rows interleaves into block-1 attention:
            # as each query tile's x2 rows land (in SBUF -- no DRAM
            # roundtrip), run the pure-DVE LN chain, stage the bf16 result
            # into that half's gather input, and kick the half-AllGather
            # that publishes it group-wide. The first gather runs
            # concurrently with block-1's second query tile.
            zb2 = [None] * NOT
            x2_sb = [None] * NOT

            def _b2_q(ntk):
                """Block-2 Q path for query tile ntk: zTq transposes of the
                own zb2 tiles plus the Q projection. Emitted from inside
                block 1 so the ntk-0 half is ready mid-block-1 and block-2
                attention can start the instant block 1 drains."""
                for n in range(ntk * 4, ntk * 4 + 4):
                    _transpose_tile(nc, pools, C, zb2[n][:, :], C["zTq"],
                                    n * P, f"q1_{n}")
                for dt_ in range(DT):
                    ps = psB.tile([P, QT], F32, tag="acc",
                                  name=f"pq1_{dt_}_{ntk}")
                    for ki in range(DT):
                        nc.tensor.matmul(
                            ps[:, :],
                            lhsT=C["wqT"][:, ki, dt_ * P:(dt_ + 1) * P],
                            rhs=C["zTq"][:, ki, ntk * QT:(ntk + 1) * QT],
                            start=(ki == 0), stop=(ki == DT - 1))
                    nc.vector.tensor_scalar(
                        out=C["qt"][:, dt_, ntk * QT:(ntk + 1) * QT],
                        in0=ps[:, :], scalar1=C["bq_col"][:, dt_:dt_ + 1],
                        scalar2=None, op0=ALU.add)

            def _b2_ln(ntk):
                for n in range(ntk * 4, ntk * 4 + 4):
                    zb2[n] = _emit_ln_tile(nc, pools, C2, None, n, 1,
                                           src_sb=x2_sb[n])
                    nc.sync.dma_start(
                        out=_rows(gin_d[ntk][:], (n % 4) * P, P),
                        in_=zb2[n][:, :])
                nc.gpsimd.collective_compute(
                    "AllGather", mybir.AluOpType.bypass,
                    replica_groups=CC_GROUPS,
                    ins=[gin_d[ntk][:].opt()], outs=[gout_d[ntk][:].opt()])
                if ntk == 0:
                    # ntk-1's Q path is deferred into block 2 (mid_cb):
                    # emitted here it would head-block the spill-phase
                    # attention behind its end-of-block-1 LN dependency
                    _b2_q(ntk)

            _build_block(nc, pools, C1, di["xs"][:], None, 0,
                         after_qt=_b2_ln, out_sb=x2_sb)
            _build_block(nc, pools, C2, None, out_d[:], 1,
                         zb_dram=[g[:] for g in gout_d], premade_zb=zb2,
                         res_sb=x2_sb, mid_cb=lambda: _b2_q(1))

    _fix_sync_waits(nc)
    return nc


_NC_CACHE = {}


def _get_nc(identity_ln=True, zero_bv=True):
    key = (identity_ln, zero_bv)
    if key not in _NC_CACHE:
        _NC_CACHE[key] = _build_program(identity_ln, zero_bv)
    return _NC_CACHE[key]


def _prep_inputs(x, a0, b0, ra0, rb0, ra1, rb1,
                 wq, bq, wk, bk, wv, bv, wo, bo):
    bf = ml_dtypes.bfloat16
    base = {
        "wqT": np.ascontiguousarray(np.asarray(wq, np.float32).T).astype(bf),
        "wkT": np.ascontiguousarray(np.asarray(wk, np.float32).T).astype(bf),
        "wvT": np.ascontiguousarray(np.asarray(wv, np.float32).T).astype(bf),
        "woT": np.ascontiguousarray(np.asarray(wo, np.float32).T).astype(bf),
        "bq_col": np.ascontiguousarray(
            np.asarray(bq, np.float32).reshape(DT, P).T),
        "bk_col": np.ascontiguousarray(
            np.asarray(bk, np.float32).reshape(DT, P).T),
        "bv_bc": np.ascontiguousarray(
            np.broadcast_to(np.asarray(bv, np.float32), (P, D))),
        "bo_row": np.asarray(bo, np.float32).reshape(1, D).copy(),
        "ra0_bc": np.ascontiguousarray(
            np.broadcast_to(np.asarray(ra0, np.float32), (P, D))),
        "rb0_bc": np.ascontiguousarray(
            np.broadcast_to(np.asarray(rb0, np.float32), (P, D))),
        "ra1_bc": np.ascontiguousarray(
            np.broadcast_to(np.asarray(ra1, np.float32), (P, D))),
        "rb1_bc": np.ascontiguousarray(
            np.broadcast_to(np.asarray(rb1, np.float32), (P, D))),
        "a0_bc": np.ascontiguousarray(
            np.broadcast_to(np.asarray(a0, np.float32), (P, D))),
        "b0_bc": np.ascontiguousarray(
            np.broadcast_to(np.asarray(b0, np.float32), (P, D))),
        "ones_in": np.ones((1, P), np.float32),
        "ident_in": np.eye(P, dtype=np.float32).astype(bf),
    }
    base["wAll"] = np.ascontiguousarray(np.concatenate(
        [base["wqT"], base["wkT"], base["wvT"], base["woT"]], axis=0))
    base["bqk_col"] = np.ascontiguousarray(
        np.concatenate([base["bq_col"], base["bk_col"]], axis=1))
    x = np.asarray(x, np.float32)
    in_maps = []
    for c in range(8):
        b, q0 = c // GRP, (c % GRP) * OWN
        m = dict(base)
        # rotate tokens so this core's output shard sits at rows 0..OWN
        m["xs"] = np.ascontiguousarray(
            np.concatenate([x[b, q0:], x[b, :q0]], axis=0))
        in_maps.append(m)
    return in_maps


def kernel(**inputs):
    identity_ln = all(
        bool(np.all(np.asarray(inputs[k], np.float32) == v))
        for k, v in (("a0", 1.0), ("b0", 0.0), ("ra0", 1.0), ("rb0", 0.0),
                     ("ra1", 1.0), ("rb1", 0.0)))
    zero_bv = all(
        bool(np.all(np.asarray(inputs[k], np.float32) == 0.0))
        for k in ("bq", "bk", "bv"))
    nc = _get_nc(identity_ln, zero_bv)
    in_maps = _prep_inputs(**inputs)
    res = run_bass_kernel_spmd(nc, in_maps, list(range(8)))
    B = inputs["x"].shape[0]
    out = np.empty((B, S, D), np.float32)
    for c in range(8):
        b, q0 = c // GRP, (c % GRP) * OWN
        out[b, q0:q0 + OWN, :] = res.results[c]["out"]
    return out


if __name__ == "__main__":
    rng = np.random.default_rng(0)
    ins = {
        "x": rng.standard_normal((2, S, D)).astype(np.float32),
        "a0": np.ones(D, np.float32), "b0": np.zeros(D, np.float32),
        "ra0": np.ones(D, np.float32), "rb0": np.zeros(D, np.float32),
        "ra1": np.ones(D, np.float32), "rb1": np.zeros(D, np.float32),
        "wq": (rng.standard_normal((D, D)) * 0.02).astype(np.float32),
        "bq": np.zeros(D, np.float32),
        "wk": (rng.standard_normal((D, D)) * 0.02).astype(np.float32),
        "bk": np.zeros(D, np.float32),
        "wv": (rng.standard_normal((D, D)) * 0.02).astype(np.float32),
        "bv": np.zeros(D, np.float32),
        "wo": (rng.standard_normal((D, D)) * 0.02).astype(np.float32),
        "bo": np.zeros(D, np.float32),
    }
    out = kernel(**ins)
    print("kernel ran, out shape", out.shape, out.dtype)
